# revision 1
# baseline (speedup 1.0000x reference)
"""Trainium2 Bass kernel: fused ViT-style attention rollout gating.

Math (per sample b):
  qkT[d, n]   = W_qk[d, :] @ x[b][:, n]          (d = 2*896: q rows then k rows)
  logits[h]   = qT_h.T @ kT_h                    ([49, 49] per head, K = head_dim = 128)
  attn[h]     = softmax(scale * logits[h])       (row-wise; no max-subtraction: |logits*scale| < 2)
  fused       = min_h attn[h]                    ([49, 49])
  rowsum[n]   = sum_m fused[n, m];  colsum[m] = sum_n fused[n, m]
  att[m]      = (colsum[m] + 1) / (49 * (rowsum[m] + 1))
  rx[b]       = x[b] * (1 + att)                 (broadcast over channels)

The reference's flat-topk masking quirk only touches global sample 0; it is
corrected exactly on the host from the device-exported `fused` matrices.

Sharding: pure data-parallel, 128 samples per core across 8 cores.
Layout per core/sub-batch (SB=16 samples): x is loaded as 7 c-tiles
[128c, 16b, 49n]; attention tiles pack 16 samples as 2 vertical partition
blocks (base 0 / 64, via PE column tiling) x 8 horizontal 49-col slots.
"""

import numpy as np
import ml_dtypes

# ---- problem constants (hardcoded per contest rules) ----
B_FULL = 1024
C = 896
N = 49                   # tokens (7x7)
NH = 7                   # heads
HD = 128                 # head dim
NCORES = 8
B_CORE = B_FULL // NCORES   # 128
SB = 16                     # samples per sub-batch
NSB = B_CORE // SB          # 8 sub-batches
CT = C // 128               # 7 contraction tiles
MT = 2 * C // 128           # 14 output d-tiles (q then k)
HF = 8 * N                  # 392 = half free width (8 horizontal samples)
FDX = SB * N                # 784
SCALE = float(HD) ** -0.5
NN = N * N                  # 2401
KEEP = NN - int(NN * 0.9)   # 241 largest kept out of topk(smallest 90%)

_CACHE = {}
LAST_RESULTS = None  # BassKernelResults of the most recent kernel() call


def _build(nsb=NSB):
    import concourse.tile as tile
    from concourse import bacc, mybir

    dt = mybir.dt
    f32 = dt.float32
    bf16 = dt.bfloat16
    AF = mybir.ActivationFunctionType
    ALU = mybir.AluOpType
    AX = mybir.AxisListType

    nc = bacc.Bacc("TRN2", target_bir_lowering=False, debug=False,
                   num_devices=NCORES)
    # x and rx travel in channel-major layout [C, B, N] (host transposes both
    # ways) so every DMA run is 16*49*4 = 3136 contiguous bytes
    x_d = nc.dram_tensor("x", [C, B_CORE, N], f32, kind="ExternalInput").ap()
    wt_d = nc.dram_tensor("wt", [C, MT * 128], bf16,
                           kind="ExternalInput").ap()
    rx_d = nc.dram_tensor("rx", [C, B_CORE, N], f32, kind="ExternalOutput").ap()
    fus_d = nc.dram_tensor("fus", [NSB, 2, N, HF], f32,
                           kind="ExternalOutput").ap()

    with tile.TileContext(nc) as tc:
        with (
            tc.tile_pool(name="w", bufs=1) as wpool,
            tc.tile_pool(name="xt", bufs=3) as xtpool,
            tc.tile_pool(name="xb", bufs=2) as xbpool,
            tc.tile_pool(name="qk", bufs=2) as qkpool,
            tc.tile_pool(name="e", bufs=1) as epool,
            tc.tile_pool(name="sm", bufs=2) as spool,
            tc.tile_pool(name="qps", bufs=2, space="PSUM") as qpspool,
            tc.tile_pool(name="aps", bufs=4, space="PSUM") as apspool,
            tc.tile_pool(name="cps", bufs=1, space="PSUM") as cpspool,
            tc.tile_pool(name="dram", bufs=2, space="DRAM") as dpool,
        ):
            # ---- one-time: weights (pre-transposed on host) + colsum ones ----
            wtb = []
            for k in range(CT):
                w = wpool.tile([128, MT * 128], bf16, tag=f"w{k}")
                nc.sync.dma_start(out=w[:], in_=wt_d[128 * k:128 * (k + 1), :])
                wtb.append(w)
            ones2 = wpool.tile([128, 2], f32, tag="ones2")
            nc.vector.memset(ones2[:], 0.0)
            nc.vector.memset(ones2[0:N, 0:1], 1.0)
            nc.vector.memset(ones2[64:64 + N, 1:2], 1.0)

            state = {}
            qkv_state = {}

            def emit_front(s):
                # ---- load x: fp32 copy (for the final multiply) via sync
                # HWDGE, bf16 copy (for the PE) via a gpsimd casting DMA.
                x_src = x_d.rearrange("(ct p) b n -> p ct b n",
                                      p=128)[:, :, SB * s:SB * (s + 1), :]
                xt = xtpool.tile([128, CT, SB, N], f32, tag="xt",
                                 name=f"xt_{s}")
                nc.sync.dma_start(out=xt[:], in_=x_src)
                xb = xbpool.tile([128, CT, FDX], bf16, tag="xb",
                                 name=f"xb_{s}")
                nc.gpsimd.dma_start(
                    out=xb[:].rearrange("p ct (b n) -> p ct b n", n=N),
                    in_=x_src)

                # ---- qkv projection: qkT[m] = W[m-tile] @ x ----
                # qk tiles carry 16 zero columns of tail padding so the
                # M=64-wide attention stationary reads below never run off
                # the end (jj=15 reads columns 735:799)
                qks = []
                for m in range(MT):
                    qk = qkpool.tile([128, FDX + 16], bf16, tag=f"qk{m}",
                                     name=f"qk{m}_{s}")
                    if s < 2:
                        # zero the stationary-overhang padding once per pool
                        # slot (bufs=2, tag-deterministic rotation); nothing
                        # ever writes these 16 columns again
                        nc.vector.memset(qk[:, FDX:], 0.0)
                    for half in range(2):
                        q = qpspool.tile([128, HF], f32, tag="qps",
                                         name=f"qps{half}_{m}_{s}",
                                         bufs=3)
                        for k in range(CT):
                            nc.tensor.matmul(
                                q[:],
                                lhsT=wtb[k][:, 128 * m:128 * (m + 1)],
                                rhs=xb[:, k, HF * half:HF * (half + 1)],
                                start=(k == 0), stop=(k == CT - 1))
                        nc.scalar.copy(
                            out=qk[:, HF * half:HF * (half + 1)], in_=q[:])
                    qks.append(qk)
                qkv_state[s] = (xt, qks)

            def emit_attn(s):
                xt, qks = qkv_state.pop(s)
                # ---- attention logits + exp + per-head normalize + min ----
                S = spool.tile([128, NH, 8], f32, tag="S", name=f"S_{s}")
                R = spool.tile([128, NH, 8], f32, tag="R", name=f"R_{s}")
                F = spool.tile([128, 8, N], f32, tag="F", name=f"F_{s}",
                               bufs=3)
                T = spool.tile([128, 8, N], f32, tag="T", name=f"T_{s}",
                               bufs=1)
                for h in range(NH):
                    A = apspool.tile([128, HF], f32, tag="A", name=f"A{h}_{s}")
                    for j in range(8):
                        # vertical block 0: samples j     -> partitions 0:49
                        # vertical block 1: samples 8 + j -> partitions 64:113
                        # (M=64 stationary: rows 49:64 / 113:128 get junk dot
                        # products from neighboring columns — finite, masked
                        # out downstream — so every PSUM row is written)
                        nc.tensor.matmul(
                            A[0:64, N * j:N * (j + 1)],
                            lhsT=qks[h][:, N * j:N * j + 64],
                            rhs=qks[NH + h][:, N * j:N * (j + 1)],
                            start=True, stop=True)
                        nc.tensor.matmul(
                            A[64:128, N * j:N * (j + 1)],
                            lhsT=qks[h][:, N * (8 + j):N * (8 + j) + 64],
                            rhs=qks[NH + h][:, N * (8 + j):N * (9 + j)],
                            start=True, stop=True)
                    E = epool.tile([128, 8, N], f32, tag=f"E{h}",
                                   name=f"E{h}_{s}")
                    nc.scalar.activation(
                        out=E[:], in_=A[:].rearrange("p (j n) -> p j n", n=N),
                        func=AF.Exp, scale=SCALE)
                    nc.vector.reduce_sum(out=S[:, h, :], in_=E[:], axis=AX.X)
                    nc.vector.reciprocal(out=R[:, h, :], in_=S[:, h, :])
                    rb = R[:, h, :].unsqueeze(2).broadcast_to([128, 8, N])
                    dst = F if h == 0 else T
                    nc.vector.tensor_tensor(
                        out=dst[:], in0=E[:], in1=rb, op=ALU.mult)
                    if h > 0:
                        nc.vector.tensor_tensor(
                            out=F[:], in0=F[:], in1=T[:], op=ALU.min)

                # ---- rowsum (free-dim reduce); all dependent DMAs are
                # deferred to the tail so they never block a queue head ----
                RS = spool.tile([128, 8], f32, tag="RS", name=f"RS_{s}",
                                bufs=3)
                nc.vector.reduce_sum(out=RS[:], in_=F[:], axis=AX.X)
                state[s] = (xt, F, RS)

            def emit_tail(s):
                # Deferred by one sub-batch: the colsum matmul below depends
                # on the DVE min chain; emitting it after the NEXT sub-batch's
                # attention matmuls keeps the PE queue from ever waiting.
                xt, F, RS = state.pop(s)

                # export fused for the host-side topk mask correction
                nc.scalar.dma_start(
                    out=fus_d[s, 0],
                    in_=F[0:N].rearrange("p j n -> p (j n)"))
                nc.scalar.dma_start(
                    out=fus_d[s, 1],
                    in_=F[64:64 + N].rearrange("p j n -> p (j n)"))
                rs_dram = dpool.tile([2, 8, N], f32, tag="rsd",
                                     name=f"rsd_{s}")
                nc.scalar.dma_start(
                    out=rs_dram[0].transpose([1, 0]), in_=RS[0:N, :])
                nc.scalar.dma_start(
                    out=rs_dram[1].transpose([1, 0]), in_=RS[64:64 + N, :])
                Rs = spool.tile([SB, N], f32, tag="Rs", name=f"Rs_{s}")
                nc.scalar.dma_start(
                    out=Rs[:],
                    in_=rs_dram[:].rearrange("k j n -> (k j) n"))
                Cp = cpspool.tile([2, HF], f32, tag="C", name=f"C_{s}")
                nc.tensor.matmul(
                    Cp[:], lhsT=ones2[:],
                    rhs=F[:].rearrange("p j n -> p (j n)"),
                    start=True, stop=True)
                Csb = spool.tile([2, 8, N], f32, tag="Csb", name=f"Csb_{s}")
                nc.scalar.copy(
                    out=Csb[:], in_=Cp[:].rearrange("p (j n) -> p j n", n=N))

                cs_dram = dpool.tile([2, 8, N], f32, tag="csd",
                                     name=f"csd_{s}")
                nc.scalar.dma_start(out=cs_dram[:], in_=Csb[:])
                Cs = spool.tile([SB, N], f32, tag="Cs", name=f"Cs_{s}")
                nc.scalar.dma_start(
                    out=Cs[:],
                    in_=cs_dram[:].rearrange("k j n -> (k j) n"))

                # ---- att + 1 = (colsum+1)/(49*(rowsum+1)) + 1 ----
                D = spool.tile([SB, N], f32, tag="D", name=f"D_{s}")
                nc.scalar.activation(out=D[:], in_=Rs[:], func=AF.Copy,
                                     scale=float(N), bias=float(N))
                nc.vector.reciprocal(out=D[:], in_=D[:])
                M1 = spool.tile([SB, N], f32, tag="M1", name=f"M1_{s}")
                nc.vector.tensor_scalar_add(M1[:], Cs[:], 1.0)
                nc.vector.tensor_tensor(
                    out=M1[:], in0=M1[:], in1=D[:], op=ALU.mult)
                nc.vector.tensor_scalar_add(M1[:], M1[:], 1.0)

                # broadcast multiplier to all 128 partitions via DRAM
                m1_dram = dpool.tile([FDX], f32, tag="m1d", name=f"m1d_{s}")
                nc.scalar.dma_start(out=m1_dram[:], in_=M1[:])
                M1b = spool.tile([128, SB, N], f32, tag="M1b",
                                 name=f"M1b_{s}")
                nc.scalar.dma_start(
                    out=M1b[:],
                    in_=m1_dram[:].rearrange("(b n) -> b n",
                                             n=N).partition_broadcast(128))

                # ---- rx = x * (1 + att), in place, then store ----
                # flat per-chunk multiplies (Pool is slow on broadcast APs),
                # 3 chunks on GpSimd / 4 on Vector
                for k in range(CT):
                    nc.vector.tensor_tensor(
                        out=xt[:, k], in0=xt[:, k], in1=M1b[:], op=ALU.mult)
                dst = rx_d.rearrange("(ct p) b n -> p ct b n",
                                     p=128)[:, :, SB * s:SB * (s + 1), :]
                nc.scalar.dma_start(out=dst, in_=xt[:])

            for s in range(nsb):
                emit_front(s)
                if s > 1:
                    emit_tail(s - 2)
                emit_attn(s)
            if nsb > 1:
                emit_tail(nsb - 2)
            emit_tail(nsb - 1)

    nc.compile()
    return nc


def _get_program(nsb=NSB):
    if nsb not in _CACHE:
        _CACHE[nsb] = _build(nsb)
    return _CACHE[nsb]


def _host_finalize(rx, x5, fused_all):
    """Exact replication of the reference's flat-topk masking quirk.

    Only global sample 0 is affected: its fused matrix is masked by the
    union of all samples' bottom-90% index sets (computed from the
    device-exported fused matrices), then its att row is rebuilt exactly.
    """
    thr = np.partition(fused_all, NN - KEEP, axis=1)[:, NN - KEEP]
    in_top = fused_all >= thr[:, None]
    zero_mask = (~in_top).any(axis=0)
    zero_mask[0] = False
    f0 = fused_all[0].copy()
    f0[zero_mask] = 0.0
    fm = f0.reshape(N, N)
    rowsum = fm.sum(axis=1)
    colsum = fm.sum(axis=0)
    att0 = (colsum + 1.0) / (N * (rowsum + 1.0))
    rx[0] = x5[0] * (1.0 + att0[None, :].astype(np.float32))
    return rx


def _par(fn, n):
    from concurrent.futures import ThreadPoolExecutor
    with ThreadPoolExecutor(max_workers=n) as ex:
        list(ex.map(fn, range(n)))


def kernel(x, W_qkv):
    from concourse.bass_utils import run_bass_kernel_spmd

    nc = _get_program()
    x5 = np.asarray(x, dtype=np.float32).reshape(B_FULL, C, N)
    wt = np.ascontiguousarray(
        np.asarray(W_qkv, dtype=np.float32)[:MT * 128].T
    ).astype(ml_dtypes.bfloat16)

    # per-core channel-major shards [C, B_CORE, N]
    shards = [np.empty((C, B_CORE, N), np.float32) for _ in range(NCORES)]
    _par(lambda c: np.copyto(
        shards[c], x5[B_CORE * c:B_CORE * (c + 1)].transpose(1, 0, 2)), NCORES)

    in_maps = [{"x": shards[c], "wt": wt} for c in range(NCORES)]
    res = run_bass_kernel_spmd(nc, in_maps, core_ids=list(range(NCORES)))
    global LAST_RESULTS
    LAST_RESULTS = res

    rx = np.empty((B_FULL, C, N), np.float32)
    fused_all = np.empty((B_FULL, NN), np.float32)

    def _gather(c):
        out = res.results[c]
        rx[B_CORE * c:B_CORE * (c + 1)] = out["rx"].transpose(1, 0, 2)
        f = out["fus"].reshape(NSB, 2, N, 8, N).transpose(0, 1, 3, 2, 4)
        fused_all[B_CORE * c:B_CORE * (c + 1)] = f.reshape(B_CORE, NN)

    _par(_gather, NCORES)

    rx = _host_finalize(rx, x5, fused_all)
    return rx.reshape(B_FULL, C, 7, 7)



# revision 5
# speedup vs baseline: 1.0695x; 1.0695x over previous
"""Trainium2 Bass kernel: fused ViT-style attention rollout gating.

Math (per sample b):
  qkT[d, n]   = W_qk[d, :] @ x[b][:, n]          (d = 2*896: q rows then k rows)
  logits[h]   = qT_h.T @ kT_h                    ([49, 49] per head, K = head_dim = 128)
  attn[h]     = softmax(scale * logits[h])       (row-wise; no max-subtraction: |logits*scale| < 2)
  fused       = min_h attn[h]                    ([49, 49])
  rowsum[n]   = sum_m fused[n, m];  colsum[m] = sum_n fused[n, m]
  att[m]      = (colsum[m] + 1) / (49 * (rowsum[m] + 1))
  rx[b]       = x[b] * (1 + att)                 (broadcast over channels)

The reference's flat-topk masking quirk only touches global sample 0; it is
corrected exactly on the host from the device-exported `fused` matrices.

Sharding: pure data-parallel, 128 samples per core across 8 cores.
Layout per core/sub-batch (SB=16 samples): x is loaded as 7 c-tiles
[128c, 16b, 49n]; attention tiles pack 16 samples as 2 vertical partition
blocks (base 0 / 64, via PE column tiling) x 8 horizontal 49-col slots.

Projection runs in fp8 (e4m3) with MatmulPerfMode.DoubleRow: W is
pre-scaled by 32 on the host, cast to fp8, and contraction is padded
896 -> 1024 so each matmul contracts 256 rows (4 k-pair steps/output).
The 1/(32*32) de-scale folds exactly into the softmax exp scale.
"""

import numpy as np
import ml_dtypes

# ---- problem constants (hardcoded per contest rules) ----
B_FULL = 1024
C = 896
N = 49                   # tokens (7x7)
NH = 7                   # heads
HD = 128                 # head dim
NCORES = 8
B_CORE = B_FULL // NCORES   # 128
SB = 16                     # samples per sub-batch
NSB = B_CORE // SB          # 8 sub-batches
CT = C // 128               # 7 contraction tiles
KT = 8                      # padded contraction tiles (fp8 DoubleRow pairs)
MT = 2 * C // 128           # 14 output d-tiles (q then k)
HF = 8 * N                  # 392 = half free width (8 horizontal samples)
FDX = SB * N                # 784
WSC = 32.0                  # host-side W pre-scale (fp8 range/precision)
SCALE = float(HD) ** -0.5
NN = N * N                  # 2401
KEEP = NN - int(NN * 0.9)   # 241 largest kept out of topk(smallest 90%)

_CACHE = {}
LAST_RESULTS = None  # BassKernelResults of the most recent kernel() call


def _build(nsb=NSB):
    import concourse.tile as tile
    from concourse import bacc, mybir

    dt = mybir.dt
    f32 = dt.float32
    bf16 = dt.bfloat16
    f8 = dt.float8e4
    AF = mybir.ActivationFunctionType
    ALU = mybir.AluOpType
    AX = mybir.AxisListType
    DR = mybir.MatmulPerfMode.DoubleRow

    nc = bacc.Bacc("TRN2", target_bir_lowering=False, debug=False,
                   num_devices=NCORES)
    # x and rx travel in channel-major layout [C, B, N] (host transposes both
    # ways) so every DMA run is 16*49*4 = 3136 contiguous bytes
    x_d = nc.dram_tensor("x", [C, B_CORE, N], f32, kind="ExternalInput").ap()
    wt_d = nc.dram_tensor("wt", [128, KT, MT * 128], f8,
                          kind="ExternalInput").ap()
    rx_d = nc.dram_tensor("rx", [C, B_CORE, N], f32, kind="ExternalOutput").ap()
    fus_d = nc.dram_tensor("fus", [NSB, 2, N, HF], bf16,
                           kind="ExternalOutput").ap()

    with tile.TileContext(nc) as tc:
        with (
            tc.tile_pool(name="w", bufs=1) as wpool,
            tc.tile_pool(name="xt", bufs=3) as xtpool,
            tc.tile_pool(name="xb", bufs=2) as xbpool,
            tc.tile_pool(name="qk", bufs=2) as qkpool,
            tc.tile_pool(name="e", bufs=1) as epool,
            tc.tile_pool(name="sm", bufs=2) as spool,
            tc.tile_pool(name="qps", bufs=2, space="PSUM") as qpspool,
            tc.tile_pool(name="aps", bufs=3, space="PSUM") as apspool,
            tc.tile_pool(name="cps", bufs=1, space="PSUM") as cpspool,
            tc.tile_pool(name="dram", bufs=2, space="DRAM") as dpool,
        ):
            # ---- one-time: fp8 weights (pre-transposed/scaled/padded on
            # host, k-tile 7 all-zero) + colsum ones ----
            w8 = wpool.tile([128, KT, MT * 128], f8, tag="w8")
            nc.sync.dma_start(out=w8[:], in_=wt_d)
            ones2 = wpool.tile([128, 2], bf16, tag="ones2")
            nc.vector.memset(ones2[:], 0.0)
            nc.vector.memset(ones2[0:N, 0:1], 1.0)
            nc.vector.memset(ones2[64:64 + N, 1:2], 1.0)

            state = {}
            qkv_state = {}

            def emit_front(s):
                # ---- load x once (fp32), cast fp32->fp8 on-chip ----
                x_src = x_d.rearrange("(ct p) b n -> p ct b n",
                                      p=128)[:, :, SB * s:SB * (s + 1), :]
                xt = xtpool.tile([128, CT, SB, N], f32, tag="xt",
                                 name=f"xt_{s}")
                nc.sync.dma_start(out=xt[:], in_=x_src)
                xb = xbpool.tile([128, KT, FDX], f8, tag="xb",
                                 name=f"xb_{s}")
                if s < 2:
                    # zero the contraction padding tile once per pool slot
                    # (bufs=2, tag-deterministic rotation)
                    nc.vector.memset(xb[:, CT, :], 0.0)
                for k in range(CT):
                    src = xt[:, k].rearrange("p b n -> p (b n)")
                    if k < 1:
                        nc.scalar.copy(out=xb[:, k, :], in_=src)
                    else:
                        nc.gpsimd.tensor_copy(out=xb[:, k, :], in_=src)

                # ---- qkv projection (fp8 DoubleRow): qkT[m] = W[m] @ x ----
                # qk tiles carry 16 zero columns of tail padding so the
                # M=64-wide attention stationary reads below never run off
                # the end (jj=15 reads columns 735:799)
                qks = []
                for m in range(MT):
                    qk = qkpool.tile([128, FDX + 16], bf16, tag=f"qk{m}",
                                     name=f"qk{m}_{s}")
                    if s < 2:
                        # zero the stationary-overhang padding once per pool
                        # slot; nothing ever writes these 16 columns again
                        nc.vector.memset(qk[:, FDX:], 0.0)
                    q = qpspool.tile([128, 2, 512], f32, tag="qps",
                                     name=f"qps_{m}_{s}")
                    for half in range(2):
                        for kp in range(KT // 2):
                            nc.tensor.matmul(
                                q[:, half, 0:HF],
                                lhsT=w8[:, 2 * kp:2 * kp + 2,
                                        128 * m:128 * (m + 1)],
                                rhs=xb[:, 2 * kp:2 * kp + 2,
                                       HF * half:HF * (half + 1)],
                                start=(kp == 0), stop=(kp == KT // 2 - 1),
                                perf_mode=DR)
                    # single PSUM->SBUF evacuation per m, engine-rotated
                    # (GPSIMD cannot read PSUM: Scalar/Vector only)
                    dst = qk[:, 0:FDX].rearrange("p (h f) -> p h f", h=2)
                    srcq = q[:, :, 0:HF]
                    if m % 2 == 0:
                        nc.scalar.copy(out=dst, in_=srcq)
                    else:
                        nc.vector.tensor_copy(out=dst, in_=srcq)
                    qks.append(qk)
                qkv_state[s] = (xt, qks)

            def emit_attn(s):
                xt, qks = qkv_state.pop(s)
                # ---- attention logits + exp + per-head normalize + min ----
                S = spool.tile([128, NH, 8], f32, tag="S", name=f"S_{s}")
                R = spool.tile([128, NH, 8], f32, tag="R", name=f"R_{s}")
                F = spool.tile([128, 8, N], bf16, tag="F", name=f"F_{s}",
                               bufs=3)
                T = spool.tile([128, 8, N], bf16, tag="T", name=f"T_{s}",
                               bufs=1)
                for h in range(NH):
                    A = apspool.tile([128, HF], f32, tag="A", name=f"A{h}_{s}")
                    for j in range(8):
                        # vertical block 0: samples j     -> partitions 0:49
                        # vertical block 1: samples 8 + j -> partitions 64:113
                        # (M=64 stationary: rows 49:64 / 113:128 get junk dot
                        # products from neighboring columns — finite, masked
                        # out downstream — so every PSUM row is written)
                        nc.tensor.matmul(
                            A[0:64, N * j:N * (j + 1)],
                            lhsT=qks[h][:, N * j:N * j + 64],
                            rhs=qks[NH + h][:, N * j:N * (j + 1)],
                            start=True, stop=True)
                        nc.tensor.matmul(
                            A[64:128, N * j:N * (j + 1)],
                            lhsT=qks[h][:, N * (8 + j):N * (8 + j) + 64],
                            rhs=qks[NH + h][:, N * (8 + j):N * (9 + j)],
                            start=True, stop=True)
                    E = epool.tile([128, 8, N], f32, tag=f"E{h}",
                                   name=f"E{h}_{s}")
                    nc.scalar.activation(
                        out=E[:], in_=A[:].rearrange("p (j n) -> p j n", n=N),
                        func=AF.Exp, scale=SCALE / (WSC * WSC))
                    nc.vector.reduce_sum(out=S[:, h, :], in_=E[:], axis=AX.X)
                    nc.vector.reciprocal(out=R[:, h, :], in_=S[:, h, :])
                    rb = R[:, h, :].unsqueeze(2).broadcast_to([128, 8, N])
                    dst = F if h == 0 else T
                    nc.vector.tensor_tensor(
                        out=dst[:], in0=E[:], in1=rb, op=ALU.mult)
                    if h > 0:
                        nc.vector.tensor_tensor(
                            out=F[:], in0=F[:], in1=T[:], op=ALU.min)

                # ---- rowsum (free-dim reduce); all dependent DMAs are
                # deferred to the tail so they never block a queue head ----
                RS = spool.tile([128, 8], f32, tag="RS", name=f"RS_{s}",
                                bufs=3)
                nc.vector.reduce_sum(out=RS[:], in_=F[:], axis=AX.X)
                state[s] = (xt, F, RS)

            def emit_tail(s):
                # Deferred by one sub-batch: the colsum matmul below depends
                # on the DVE min chain; emitting it after the NEXT sub-batch's
                # attention matmuls keeps the PE queue from ever waiting.
                xt, F, RS = state.pop(s)

                # export fused for the host-side topk mask correction
                nc.scalar.dma_start(
                    out=fus_d[s, 0],
                    in_=F[0:N].rearrange("p j n -> p (j n)"))
                nc.scalar.dma_start(
                    out=fus_d[s, 1],
                    in_=F[64:64 + N].rearrange("p j n -> p (j n)"))
                rs_dram = dpool.tile([2, 8, N], f32, tag="rsd",
                                     name=f"rsd_{s}")
                nc.scalar.dma_start(
                    out=rs_dram[0].transpose([1, 0]), in_=RS[0:N, :])
                nc.scalar.dma_start(
                    out=rs_dram[1].transpose([1, 0]), in_=RS[64:64 + N, :])
                Rs = spool.tile([SB, N], f32, tag="Rs", name=f"Rs_{s}")
                nc.scalar.dma_start(
                    out=Rs[:],
                    in_=rs_dram[:].rearrange("k j n -> (k j) n"))
                Cp = cpspool.tile([2, HF], f32, tag="C", name=f"C_{s}")
                nc.tensor.matmul(
                    Cp[:], lhsT=ones2[:],
                    rhs=F[:].rearrange("p j n -> p (j n)"),
                    start=True, stop=True)
                Csb = spool.tile([2, 8, N], f32, tag="Csb", name=f"Csb_{s}")
                nc.scalar.copy(
                    out=Csb[:], in_=Cp[:].rearrange("p (j n) -> p j n", n=N))

                cs_dram = dpool.tile([2, 8, N], f32, tag="csd",
                                     name=f"csd_{s}")
                nc.scalar.dma_start(out=cs_dram[:], in_=Csb[:])
                Cs = spool.tile([SB, N], f32, tag="Cs", name=f"Cs_{s}")
                nc.scalar.dma_start(
                    out=Cs[:],
                    in_=cs_dram[:].rearrange("k j n -> (k j) n"))

                # ---- att + 1 = (colsum+1)/(49*(rowsum+1)) + 1 ----
                D = spool.tile([SB, N], f32, tag="D", name=f"D_{s}")
                nc.scalar.activation(out=D[:], in_=Rs[:], func=AF.Copy,
                                     scale=float(N), bias=float(N))
                nc.vector.reciprocal(out=D[:], in_=D[:])
                M1 = spool.tile([SB, N], f32, tag="M1", name=f"M1_{s}")
                nc.vector.tensor_scalar_add(M1[:], Cs[:], 1.0)
                nc.vector.tensor_tensor(
                    out=M1[:], in0=M1[:], in1=D[:], op=ALU.mult)
                nc.vector.tensor_scalar_add(M1[:], M1[:], 1.0)

                # broadcast multiplier to all 128 partitions via DRAM
                m1_dram = dpool.tile([FDX], f32, tag="m1d", name=f"m1d_{s}")
                nc.scalar.dma_start(out=m1_dram[:], in_=M1[:])
                M1b = spool.tile([128, SB, N], f32, tag="M1b",
                                 name=f"M1b_{s}")
                nc.scalar.dma_start(
                    out=M1b[:],
                    in_=m1_dram[:].rearrange("(b n) -> b n",
                                             n=N).partition_broadcast(128))

                # ---- rx = x * (1 + att), in place, then store ----
                # flat per-chunk multiplies split between Vector and GpSimd
                for k in range(CT):
                    eng = nc.vector if k < 2 else nc.gpsimd
                    eng.tensor_tensor(
                        out=xt[:, k], in0=xt[:, k], in1=M1b[:], op=ALU.mult)
                dst = rx_d.rearrange("(ct p) b n -> p ct b n",
                                     p=128)[:, :, SB * s:SB * (s + 1), :]
                nc.scalar.dma_start(out=dst, in_=xt[:])

            for s in range(nsb):
                emit_front(s)
                if s > 1:
                    emit_tail(s - 2)
                emit_attn(s)
            if nsb > 1:
                emit_tail(nsb - 2)
            emit_tail(nsb - 1)

    nc.compile()
    return nc


def _get_program(nsb=NSB):
    if nsb not in _CACHE:
        _CACHE[nsb] = _build(nsb)
    return _CACHE[nsb]


def _host_finalize(rx, x5, fused_all):
    """Exact replication of the reference's flat-topk masking quirk.

    Only global sample 0 is affected: its fused matrix is masked by the
    union of all samples' bottom-90% index sets (computed from the
    device-exported fused matrices), then its att row is rebuilt exactly.
    """
    thr = np.partition(fused_all, NN - KEEP, axis=1)[:, NN - KEEP]
    in_top = fused_all >= thr[:, None]
    zero_mask = (~in_top).any(axis=0)
    zero_mask[0] = False
    f0 = fused_all[0].copy()
    f0[zero_mask] = 0.0
    fm = f0.reshape(N, N)
    rowsum = fm.sum(axis=1)
    colsum = fm.sum(axis=0)
    att0 = (colsum + 1.0) / (N * (rowsum + 1.0))
    rx[0] = x5[0] * (1.0 + att0[None, :].astype(np.float32))
    return rx


def _par(fn, n):
    from concurrent.futures import ThreadPoolExecutor
    with ThreadPoolExecutor(max_workers=n) as ex:
        list(ex.map(fn, range(n)))


def _prep_w(W_qkv):
    """[2C, C] -> fp8 [128, KT, MT*128]: transpose, scale, pad k-tiles."""
    wt = np.zeros((128 * KT, MT * 128), np.float32)
    wt[:C] = np.asarray(W_qkv, dtype=np.float32)[:MT * 128].T * WSC
    np.clip(wt, -240.0, 240.0, out=wt)
    return np.ascontiguousarray(
        wt.reshape(KT, 128, MT * 128).transpose(1, 0, 2)
    ).astype(ml_dtypes.float8_e4m3)


def kernel(x, W_qkv):
    from concourse.bass_utils import run_bass_kernel_spmd

    nc = _get_program()
    x5 = np.asarray(x, dtype=np.float32).reshape(B_FULL, C, N)
    wt = _prep_w(W_qkv)

    # per-core channel-major shards [C, B_CORE, N]
    shards = [np.empty((C, B_CORE, N), np.float32) for _ in range(NCORES)]
    _par(lambda c: np.copyto(
        shards[c], x5[B_CORE * c:B_CORE * (c + 1)].transpose(1, 0, 2)), NCORES)

    in_maps = [{"x": shards[c], "wt": wt} for c in range(NCORES)]
    res = run_bass_kernel_spmd(nc, in_maps, core_ids=list(range(NCORES)))
    global LAST_RESULTS
    LAST_RESULTS = res

    rx = np.empty((B_FULL, C, N), np.float32)
    fused_all = np.empty((B_FULL, NN), np.float32)

    def _gather(c):
        out = res.results[c]
        rx[B_CORE * c:B_CORE * (c + 1)] = out["rx"].transpose(1, 0, 2)
        f = out["fus"].astype(np.float32).reshape(
            NSB, 2, N, 8, N).transpose(0, 1, 3, 2, 4)
        fused_all[B_CORE * c:B_CORE * (c + 1)] = f.reshape(B_CORE, NN)

    _par(_gather, NCORES)

    rx = _host_finalize(rx, x5, fused_all)
    return rx.reshape(B_FULL, C, 7, 7)


# revision 11
# speedup vs baseline: 1.2796x; 1.1964x over previous
"""Trainium2 Bass kernel: fused ViT-style attention rollout gating.

Math (per sample b):
  qkT[d, n]   = W_qk[d, :] @ x[b][:, n]          (d = 2*896: q rows then k rows)
  logits[h]   = qT_h.T @ kT_h                    ([49, 49] per head, K = head_dim = 128)
  attn[h]     = softmax(scale * logits[h])       (row-wise; no max-subtraction: |logits*scale| < 2)
  fused       = min_h attn[h]                    ([49, 49])
  rowsum[n]   = sum_m fused[n, m];  colsum[m] = sum_n fused[n, m]
  att[m]      = (colsum[m] + 1) / (49 * (rowsum[m] + 1))
  rx[b]       = x[b] * (1 + att)                 (broadcast over channels)

The reference's flat-topk masking quirk only touches global sample 0; it is
corrected exactly on the host from the device-exported `fused` matrices.

Sharding: pure data-parallel, 128 samples per core across 8 cores.
Layout per core/sub-batch (SB=16 samples): x is loaded as 7 c-tiles
[128c, 16b, 49n]; attention tiles pack 16 samples as 2 vertical partition
blocks (base 0 / 64, via PE column tiling) x 8 horizontal 49-col slots.

Projection runs in fp8 (e4m3) with MatmulPerfMode.DoubleRow: W is
pre-scaled by 32 on the host, cast to fp8, and contraction is padded
896 -> 1024 so each matmul contracts 256 rows (4 k-pair steps/output).
The 1/(32*32) de-scale folds exactly into the softmax exp scale.
"""

import numpy as np
import ml_dtypes

# ---- problem constants (hardcoded per contest rules) ----
B_FULL = 1024
C = 896
N = 49                   # tokens (7x7)
NH = 7                   # heads
HD = 128                 # head dim
NCORES = 8
B_CORE = B_FULL // NCORES   # 128
SB = 16                     # samples per sub-batch
NSB = B_CORE // SB          # 8 sub-batches
CT = C // 128               # 7 contraction tiles
KT = 8                      # padded contraction tiles (fp8 DoubleRow pairs)
MT = 2 * C // 128           # 14 output d-tiles (q then k)
HF = 8 * N                  # 392 = half free width (8 horizontal samples)
FDX = SB * N                # 784
WSC = 32.0                  # host-side W pre-scale (fp8 range/precision)
SCALE = float(HD) ** -0.5
NN = N * N                  # 2401
KEEP = NN - int(NN * 0.9)   # 241 largest kept out of topk(smallest 90%)

_CACHE = {}
LAST_RESULTS = None  # BassKernelResults of the most recent kernel() call


def _build(nsb=NSB):
    import concourse.tile as tile
    from concourse import bacc, mybir

    dt = mybir.dt
    f32 = dt.float32
    bf16 = dt.bfloat16
    f8 = dt.float8e4
    AF = mybir.ActivationFunctionType
    ALU = mybir.AluOpType
    AX = mybir.AxisListType
    DR = mybir.MatmulPerfMode.DoubleRow

    nc = bacc.Bacc("TRN2", target_bir_lowering=False, debug=False,
                   num_devices=NCORES)
    # x and rx travel in channel-major layout [C, B, N] (host transposes both
    # ways) so every DMA run is 16*49*4 = 3136 contiguous bytes
    x_d = nc.dram_tensor("x", [C, B_CORE, N], f32, kind="ExternalInput").ap()
    wt_d = nc.dram_tensor("wt", [128, KT, MT * 128], f8,
                          kind="ExternalInput").ap()
    rx_d = nc.dram_tensor("rx", [C, B_CORE, N], f32, kind="ExternalOutput").ap()
    fus_d = nc.dram_tensor("fus", [NSB, 2, N, HF], bf16,
                           kind="ExternalOutput").ap()

    with tile.TileContext(nc) as tc:
        with (
            tc.tile_pool(name="w", bufs=1) as wpool,
            tc.tile_pool(name="xt", bufs=4) as xtpool,
            tc.tile_pool(name="xb", bufs=2) as xbpool,
            tc.tile_pool(name="qk", bufs=2) as qkpool,
            tc.tile_pool(name="e", bufs=1) as epool,
            tc.tile_pool(name="sm", bufs=2) as spool,
            tc.tile_pool(name="qps", bufs=2, space="PSUM") as qpspool,
            tc.tile_pool(name="aps", bufs=3, space="PSUM") as apspool,
            tc.tile_pool(name="cps", bufs=1, space="PSUM") as cpspool,
            tc.tile_pool(name="dram", bufs=2, space="DRAM") as dpool,
        ):
            # ---- one-time: fp8 weights (pre-transposed/scaled/padded on
            # host, k-tile 7 all-zero) + colsum ones ----
            w8 = wpool.tile([128, KT, MT * 128], f8, tag="w8")
            nc.sync.dma_start(out=w8[:], in_=wt_d)
            ones2 = wpool.tile([128, 2], bf16, tag="ones2")
            nc.vector.memset(ones2[:], 0.0)
            nc.vector.memset(ones2[0:N, 0:1], 1.0)
            nc.vector.memset(ones2[64:64 + N, 1:2], 1.0)

            state = {}
            qkv_state = {}

            def emit_front(s):
                # ---- load x once (fp32), cast fp32->fp8 on-chip ----
                x_src = x_d.rearrange("(ct p) b n -> p ct b n",
                                      p=128)[:, :, SB * s:SB * (s + 1), :]
                xt = xtpool.tile([128, CT, SB, N], f32, tag="xt",
                                 name=f"xt_{s}")
                nc.sync.dma_start(out=xt[:], in_=x_src)
                xb = xbpool.tile([128, KT, FDX], f8, tag="xb",
                                 name=f"xb_{s}")
                if s < 2:
                    # zero the contraction padding tile once per pool slot
                    # (bufs=2, tag-deterministic rotation)
                    nc.vector.memset(xb[:, CT, :], 0.0)
                for k in range(CT):
                    src = xt[:, k].rearrange("p b n -> p (b n)")
                    nc.vector.tensor_copy(out=xb[:, k, :], in_=src)

                # ---- qkv projection (fp8 DoubleRow): qkT[m] = W[m] @ x ----
                # qk tiles carry 16 zero columns of tail padding so the
                # M=64-wide attention stationary reads below never run off
                # the end (jj=15 reads columns 735:799)
                qks = []
                for m in range(MT):
                    qk = qkpool.tile([128, FDX + 16], bf16, tag=f"qk{m}",
                                     name=f"qk{m}_{s}")
                    if s < 2:
                        # zero the stationary-overhang padding once per pool
                        # slot; nothing ever writes these 16 columns again
                        nc.vector.memset(qk[:, FDX:], 0.0)
                    q = qpspool.tile([128, 2, 512], f32, tag="qps",
                                     name=f"qps_{m}_{s}")
                    for half in range(2):
                        for kp in range(KT // 2):
                            nc.tensor.matmul(
                                q[:, half, 0:HF],
                                lhsT=w8[:, 2 * kp:2 * kp + 2,
                                        128 * m:128 * (m + 1)],
                                rhs=xb[:, 2 * kp:2 * kp + 2,
                                       HF * half:HF * (half + 1)],
                                start=(kp == 0), stop=(kp == KT // 2 - 1),
                                perf_mode=DR)
                    # single PSUM->SBUF evacuation per m on Scalar
                    # (GPSIMD cannot read PSUM)
                    dst = qk[:, 0:FDX].rearrange("p (h f) -> p h f", h=2)
                    nc.scalar.copy(out=dst, in_=q[:, :, 0:HF])
                    qks.append(qk)
                qkv_state[s] = (xt, qks)

            def emit_attn(s):
                xt, qks = qkv_state.pop(s)
                # ---- attention logits + exp + per-head normalize + min ----
                S = spool.tile([128, NH, 8], f32, tag="S", name=f"S_{s}")
                R = spool.tile([128, NH, 8], f32, tag="R", name=f"R_{s}")
                F = spool.tile([128, 8, N], bf16, tag="F", name=f"F_{s}",
                               bufs=3)
                T = spool.tile([128, 8, N], bf16, tag="T", name=f"T_{s}",
                               bufs=1)
                for h in range(NH):
                    A = apspool.tile([128, HF], f32, tag="A", name=f"A{h}_{s}")
                    for j in range(8):
                        # vertical block 0: samples j     -> partitions 0:49
                        # vertical block 1: samples 8 + j -> partitions 64:113
                        # (M=64 stationary: rows 49:64 / 113:128 get junk dot
                        # products from neighboring columns — finite, masked
                        # out downstream — so every PSUM row is written)
                        nc.tensor.matmul(
                            A[0:64, N * j:N * (j + 1)],
                            lhsT=qks[h][:, N * j:N * j + 64],
                            rhs=qks[NH + h][:, N * j:N * (j + 1)],
                            start=True, stop=True)
                        nc.tensor.matmul(
                            A[64:128, N * j:N * (j + 1)],
                            lhsT=qks[h][:, N * (8 + j):N * (8 + j) + 64],
                            rhs=qks[NH + h][:, N * (8 + j):N * (9 + j)],
                            start=True, stop=True)
                    E = epool.tile([128, 8, N], f32, tag=f"E{h}",
                                   name=f"E{h}_{s}")
                    nc.scalar.activation(
                        out=E[:], in_=A[:].rearrange("p (j n) -> p j n", n=N),
                        func=AF.Exp, scale=SCALE / (WSC * WSC))
                    nc.vector.reduce_sum(out=S[:, h, :], in_=E[:], axis=AX.X)
                    nc.vector.reciprocal(out=R[:, h, :], in_=S[:, h, :])
                    rb = R[:, h, :].unsqueeze(2).broadcast_to([128, 8, N])
                    dst = F if h == 0 else T
                    nc.vector.tensor_tensor(
                        out=dst[:], in0=E[:], in1=rb, op=ALU.mult)
                    if h > 0:
                        nc.vector.tensor_tensor(
                            out=F[:], in0=F[:], in1=T[:], op=ALU.min)

                # ---- rowsum (free-dim reduce); all dependent DMAs are
                # deferred to the tail so they never block a queue head ----
                RS = spool.tile([128, 8], f32, tag="RS", name=f"RS_{s}",
                                bufs=3)
                nc.vector.reduce_sum(out=RS[:], in_=F[:], axis=AX.X)
                state[s] = (xt, F, RS)

            def emit_tail(s):
                # Deferred by one sub-batch: the colsum matmul below depends
                # on the DVE min chain; emitting it after the NEXT sub-batch's
                # attention matmuls keeps the PE queue from ever waiting.
                xt, F, RS = state.pop(s)

                # export fused for the host-side topk mask correction
                nc.scalar.dma_start(
                    out=fus_d[s, 0],
                    in_=F[0:N].rearrange("p j n -> p (j n)"))
                nc.scalar.dma_start(
                    out=fus_d[s, 1],
                    in_=F[64:64 + N].rearrange("p j n -> p (j n)"))
                rs_dram = dpool.tile([2, 8, N], f32, tag="rsd",
                                     name=f"rsd_{s}")
                nc.scalar.dma_start(
                    out=rs_dram[0].transpose([1, 0]), in_=RS[0:N, :])
                nc.scalar.dma_start(
                    out=rs_dram[1].transpose([1, 0]), in_=RS[64:64 + N, :])
                Rs = spool.tile([SB, N], f32, tag="Rs", name=f"Rs_{s}")
                nc.scalar.dma_start(
                    out=Rs[:],
                    in_=rs_dram[:].rearrange("k j n -> (k j) n"))
                Cp = cpspool.tile([2, HF], f32, tag="C", name=f"C_{s}")
                nc.tensor.matmul(
                    Cp[:], lhsT=ones2[:],
                    rhs=F[:].rearrange("p j n -> p (j n)"),
                    start=True, stop=True)
                Csb = spool.tile([2, 8, N], f32, tag="Csb", name=f"Csb_{s}")
                nc.scalar.copy(
                    out=Csb[:], in_=Cp[:].rearrange("p (j n) -> p j n", n=N))

                cs_dram = dpool.tile([2, 8, N], f32, tag="csd",
                                     name=f"csd_{s}")
                nc.scalar.dma_start(out=cs_dram[:], in_=Csb[:])
                Cs = spool.tile([SB, N], f32, tag="Cs", name=f"Cs_{s}")
                nc.scalar.dma_start(
                    out=Cs[:],
                    in_=cs_dram[:].rearrange("k j n -> (k j) n"))

                # ---- att + 1 = (colsum+1)/(49*(rowsum+1)) + 1 ----
                D = spool.tile([SB, N], f32, tag="D", name=f"D_{s}")
                nc.scalar.activation(out=D[:], in_=Rs[:], func=AF.Copy,
                                     scale=float(N), bias=float(N))
                nc.vector.reciprocal(out=D[:], in_=D[:])
                M1 = spool.tile([SB, N], f32, tag="M1", name=f"M1_{s}")
                nc.vector.tensor_scalar_add(M1[:], Cs[:], 1.0)
                nc.vector.tensor_tensor(
                    out=M1[:], in0=M1[:], in1=D[:], op=ALU.mult)
                nc.vector.tensor_scalar_add(M1[:], M1[:], 1.0)

                # broadcast multiplier to all 128 partitions via DRAM
                m1_dram = dpool.tile([FDX], f32, tag="m1d", name=f"m1d_{s}")
                nc.scalar.dma_start(out=m1_dram[:], in_=M1[:])
                M1b = spool.tile([128, SB, N], f32, tag="M1b",
                                 name=f"M1b_{s}")
                nc.scalar.dma_start(
                    out=M1b[:],
                    in_=m1_dram[:].rearrange("(b n) -> b n",
                                             n=N).partition_broadcast(128))

                # ---- rx = x * (1 + att), in place, then store ----
                # flat per-chunk multiplies split between Vector and GpSimd
                for k in range(CT):
                    nc.gpsimd.tensor_tensor(
                        out=xt[:, k], in0=xt[:, k], in1=M1b[:], op=ALU.mult)
                dst = rx_d.rearrange("(ct p) b n -> p ct b n",
                                     p=128)[:, :, SB * s:SB * (s + 1), :]
                nc.scalar.dma_start(out=dst, in_=xt[:])

            for s in range(nsb):
                emit_front(s)
                if s > 1:
                    emit_tail(s - 2)
                emit_attn(s)
            if nsb > 1:
                emit_tail(nsb - 2)
            emit_tail(nsb - 1)

    nc.compile()
    return nc


def _get_program(nsb=NSB):
    if nsb not in _CACHE:
        _CACHE[nsb] = _build(nsb)
    return _CACHE[nsb]


def _host_finalize(rx, x5, fused_all):
    """Exact replication of the reference's flat-topk masking quirk.

    Only global sample 0 is affected: its fused matrix is masked by the
    union of all samples' bottom-90% index sets (computed from the
    device-exported fused matrices), then its att row is rebuilt exactly.
    """
    thr = np.partition(fused_all, NN - KEEP, axis=1)[:, NN - KEEP]
    in_top = fused_all >= thr[:, None]
    zero_mask = (~in_top).any(axis=0)
    zero_mask[0] = False
    f0 = fused_all[0].copy()
    f0[zero_mask] = 0.0
    fm = f0.reshape(N, N)
    rowsum = fm.sum(axis=1)
    colsum = fm.sum(axis=0)
    att0 = (colsum + 1.0) / (N * (rowsum + 1.0))
    rx[0] = x5[0] * (1.0 + att0[None, :].astype(np.float32))
    return rx


def _par(fn, n):
    from concurrent.futures import ThreadPoolExecutor
    with ThreadPoolExecutor(max_workers=n) as ex:
        list(ex.map(fn, range(n)))


def _prep_w(W_qkv):
    """[2C, C] -> fp8 [128, KT, MT*128]: transpose, scale, pad k-tiles."""
    wt = np.zeros((128 * KT, MT * 128), np.float32)
    wt[:C] = np.asarray(W_qkv, dtype=np.float32)[:MT * 128].T * WSC
    np.clip(wt, -240.0, 240.0, out=wt)
    return np.ascontiguousarray(
        wt.reshape(KT, 128, MT * 128).transpose(1, 0, 2)
    ).astype(ml_dtypes.float8_e4m3)


def kernel(x, W_qkv):
    from concourse.bass_utils import run_bass_kernel_spmd

    nc = _get_program()
    x5 = np.asarray(x, dtype=np.float32).reshape(B_FULL, C, N)
    wt = _prep_w(W_qkv)

    # per-core channel-major shards [C, B_CORE, N]
    shards = [np.empty((C, B_CORE, N), np.float32) for _ in range(NCORES)]
    _par(lambda c: np.copyto(
        shards[c], x5[B_CORE * c:B_CORE * (c + 1)].transpose(1, 0, 2)), NCORES)

    in_maps = [{"x": shards[c], "wt": wt} for c in range(NCORES)]
    res = run_bass_kernel_spmd(nc, in_maps, core_ids=list(range(NCORES)))
    global LAST_RESULTS
    LAST_RESULTS = res

    rx = np.empty((B_FULL, C, N), np.float32)
    fused_all = np.empty((B_FULL, NN), np.float32)

    def _gather(c):
        out = res.results[c]
        rx[B_CORE * c:B_CORE * (c + 1)] = out["rx"].transpose(1, 0, 2)
        f = out["fus"].astype(np.float32).reshape(
            NSB, 2, N, 8, N).transpose(0, 1, 3, 2, 4)
        fused_all[B_CORE * c:B_CORE * (c + 1)] = f.reshape(B_CORE, NN)

    _par(_gather, NCORES)

    rx = _host_finalize(rx, x5, fused_all)
    return rx.reshape(B_FULL, C, 7, 7)


# revision 17
# speedup vs baseline: 1.4336x; 1.1203x over previous
"""Trainium2 Bass kernel: fused ViT-style attention rollout gating.

Math (per sample b):
  qkT[d, n]   = W_qk[d, :] @ x[b][:, n]          (d = 2*896: q rows then k rows)
  logits[h]   = qT_h.T @ kT_h                    ([49, 49] per head, K = head_dim = 128)
  attn[h]     = softmax(scale * logits[h])       (row-wise; no max-subtraction: |logits*scale| < 2)
  fused       = min_h attn[h]                    ([49, 49])
  rowsum[n]   = sum_m fused[n, m];  colsum[m] = sum_n fused[n, m]
  att[m]      = (colsum[m] + 1) / (49 * (rowsum[m] + 1))
  rx[b]       = x[b] * (1 + att)                 (broadcast over channels)

The reference's flat-topk masking quirk only touches global sample 0; it is
corrected exactly on the host from the device-exported `fused` matrices.

Sharding: pure data-parallel, 128 samples per core across 8 cores.
Layout per core/sub-batch (SB=16 samples): x is loaded as 7 c-tiles
[128c, 16b, 49n]; attention tiles pack 16 samples as 2 vertical partition
blocks (base 0 / 64, via PE column tiling) x 8 horizontal 49-col slots.

Projection runs in fp8 (e4m3) with MatmulPerfMode.DoubleRow: W is
pre-scaled by 32 on the host, cast to fp8, and contraction is padded
896 -> 1024 so each matmul contracts 256 rows (4 k-pair steps/output).
The 1/(32*32) de-scale folds exactly into the softmax exp scale.
"""

import numpy as np
import ml_dtypes

# ---- problem constants (hardcoded per contest rules) ----
B_FULL = 1024
C = 896
N = 49                   # tokens (7x7)
NH = 7                   # heads
HD = 128                 # head dim
NCORES = 8
B_CORE = B_FULL // NCORES   # 128
SB = 16                     # samples per sub-batch
NSB = B_CORE // SB          # 8 sub-batches
CT = C // 128               # 7 contraction tiles
KT = 8                      # padded contraction tiles (fp8 DoubleRow pairs)
MT = 2 * C // 128           # 14 output d-tiles (q then k)
HF = 8 * N                  # 392 = half free width (8 horizontal samples)
FDX = SB * N                # 784
WSC = 32.0                  # host-side W pre-scale (fp8 range/precision)
SCALE = float(HD) ** -0.5
NN = N * N                  # 2401
KEEP = NN - int(NN * 0.9)   # 241 largest kept out of topk(smallest 90%)

_CACHE = {}
LAST_RESULTS = None  # BassKernelResults of the most recent kernel() call


def _build(nsb=NSB):
    import concourse.tile as tile
    from concourse import bacc, mybir

    dt = mybir.dt
    f32 = dt.float32
    bf16 = dt.bfloat16
    f8 = dt.float8e4
    AF = mybir.ActivationFunctionType
    ALU = mybir.AluOpType
    AX = mybir.AxisListType
    DR = mybir.MatmulPerfMode.DoubleRow

    nc = bacc.Bacc("TRN2", target_bir_lowering=False, debug=False,
                   num_devices=NCORES)
    # x and rx travel in channel-major layout [C, B, N] (host transposes both
    # ways) so every DMA run is 16*49*4 = 3136 contiguous bytes
    x_d = nc.dram_tensor("x", [C, B_CORE, N], f32, kind="ExternalInput").ap()
    wt_d = nc.dram_tensor("wt", [128, KT, MT * 128], f8,
                          kind="ExternalInput").ap()
    rx_d = nc.dram_tensor("rx", [C, B_CORE, N], f32, kind="ExternalOutput").ap()
    fus_d = nc.dram_tensor("fus", [NSB, 2, N, HF], bf16,
                           kind="ExternalOutput").ap()

    with tile.TileContext(nc) as tc:
        with (
            tc.tile_pool(name="w", bufs=1) as wpool,
            tc.tile_pool(name="xt", bufs=4) as xtpool,
            tc.tile_pool(name="xb", bufs=2) as xbpool,
            tc.tile_pool(name="qk", bufs=2) as qkpool,
            tc.tile_pool(name="e", bufs=1) as epool,
            tc.tile_pool(name="sm", bufs=2) as spool,
            tc.tile_pool(name="qps", bufs=2, space="PSUM") as qpspool,
            tc.tile_pool(name="aps", bufs=3, space="PSUM") as apspool,
            tc.tile_pool(name="cps", bufs=1, space="PSUM") as cpspool,
            tc.tile_pool(name="dram", bufs=2, space="DRAM") as dpool,
        ):
            # ---- one-time: fp8 weights (pre-transposed/scaled/padded on
            # host, k-tile 7 all-zero) + colsum ones ----
            w8 = wpool.tile([128, KT, MT * 128], f8, tag="w8")
            nc.sync.dma_start(out=w8[:], in_=wt_d)
            ones2 = wpool.tile([128, 2], bf16, tag="ones2")
            nc.vector.memset(ones2[:], 0.0)
            nc.vector.memset(ones2[0:N, 0:1], 1.0)
            nc.vector.memset(ones2[64:64 + N, 1:2], 1.0)

            state = {}
            qkv_state = {}

            def emit_front(s):
                # ---- load x once (fp32->bf16 cast DMA), then bf16->fp8 ----
                x_src = x_d.rearrange("(ct p) b n -> p ct b n",
                                      p=128)[:, :, SB * s:SB * (s + 1), :]
                xt = xtpool.tile([128, CT, SB, N], bf16, tag="xt",
                                 name=f"xt_{s}")
                nc.gpsimd.dma_start(out=xt[:], in_=x_src)
                xb = xbpool.tile([128, KT, FDX], f8, tag="xb",
                                 name=f"xb_{s}")
                if s < 2:
                    # zero the contraction padding tile once per pool slot
                    # (bufs=2, tag-deterministic rotation)
                    nc.vector.memset(xb[:, CT, :], 0.0)
                for k in range(CT):
                    src = xt[:, k].rearrange("p b n -> p (b n)")
                    nc.vector.tensor_copy(out=xb[:, k, :], in_=src)

                # ---- qkv projection (fp8 DoubleRow): qkT[m] = W[m] @ x ----
                # qk tiles carry 16 zero columns of tail padding so the
                # M=64-wide attention stationary reads below never run off
                # the end (jj=15 reads columns 735:799)
                qks = []
                for m in range(MT):
                    qk = qkpool.tile([128, FDX + 16], bf16, tag=f"qk{m}",
                                     name=f"qk{m}_{s}")
                    if s < 2:
                        # zero the stationary-overhang padding once per pool
                        # slot; nothing ever writes these 16 columns again
                        nc.vector.memset(qk[:, FDX:], 0.0)
                    q = qpspool.tile([128, 2, 512], f32, tag="qps",
                                     name=f"qps_{m}_{s}")
                    for half in range(2):
                        for kp in range(KT // 2):
                            nc.tensor.matmul(
                                q[:, half, 0:HF],
                                lhsT=w8[:, 2 * kp:2 * kp + 2,
                                        128 * m:128 * (m + 1)],
                                rhs=xb[:, 2 * kp:2 * kp + 2,
                                       HF * half:HF * (half + 1)],
                                start=(kp == 0), stop=(kp == KT // 2 - 1),
                                perf_mode=DR)
                    # single PSUM->SBUF evacuation per m on Scalar
                    # (GPSIMD cannot read PSUM)
                    dst = qk[:, 0:FDX].rearrange("p (h f) -> p h f", h=2)
                    nc.scalar.copy(out=dst, in_=q[:, :, 0:HF])
                    qks.append(qk)
                qkv_state[s] = (xt, qks)

            def emit_attn(s):
                xt, qks = qkv_state.pop(s)
                # ---- attention logits + exp + per-head normalize + min ----
                S = spool.tile([128, NH, 8], bf16, tag="S", name=f"S_{s}")
                R = spool.tile([128, NH, 8], bf16, tag="R", name=f"R_{s}")
                F = spool.tile([128, 8, N], bf16, tag="F", name=f"F_{s}",
                               bufs=3)
                T = spool.tile([128, 8, N], bf16, tag="T", name=f"T_{s}",
                               bufs=1)
                for h in range(NH):
                    A = apspool.tile([128, HF], f32, tag="A", name=f"A{h}_{s}")
                    for j in range(8):
                        # vertical block 0: samples j     -> partitions 0:49
                        # vertical block 1: samples 8 + j -> partitions 64:113
                        # (M=64 stationary: rows 49:64 / 113:128 get junk dot
                        # products from neighboring columns — finite, masked
                        # out downstream — so every PSUM row is written)
                        nc.tensor.matmul(
                            A[0:64, N * j:N * (j + 1)],
                            lhsT=qks[h][:, N * j:N * j + 64],
                            rhs=qks[NH + h][:, N * j:N * (j + 1)],
                            start=True, stop=True)
                        nc.tensor.matmul(
                            A[64:128, N * j:N * (j + 1)],
                            lhsT=qks[h][:, N * (8 + j):N * (8 + j) + 64],
                            rhs=qks[NH + h][:, N * (8 + j):N * (9 + j)],
                            start=True, stop=True)
                    E = epool.tile([128, 8, N], bf16, tag=f"E{h}",
                                   name=f"E{h}_{s}")
                    nc.scalar.activation(
                        out=E[:], in_=A[:].rearrange("p (j n) -> p j n", n=N),
                        func=AF.Exp, scale=SCALE / (WSC * WSC))
                    with nc.allow_low_precision(
                            reason="49-term softmax sums; 2e-2 gate"):
                        nc.vector.reduce_sum(out=S[:, h, :], in_=E[:],
                                             axis=AX.X)
                        nc.vector.reciprocal(out=R[:, h, :], in_=S[:, h, :])
                    rb = R[:, h, :].unsqueeze(2).broadcast_to([128, 8, N])
                    dst = F if h == 0 else T
                    nc.vector.tensor_tensor(
                        out=dst[:], in0=E[:], in1=rb, op=ALU.mult)
                    if h > 0:
                        nc.vector.tensor_tensor(
                            out=F[:], in0=F[:], in1=T[:], op=ALU.min)

                # ---- rowsum (free-dim reduce); all dependent DMAs are
                # deferred to the tail so they never block a queue head ----
                RS = spool.tile([128, 8], f32, tag="RS", name=f"RS_{s}",
                                bufs=3)
                nc.vector.reduce_sum(out=RS[:], in_=F[:], axis=AX.X)
                state[s] = (xt, F, RS)

            def emit_tail(s):
                # Deferred by one sub-batch: the colsum matmul below depends
                # on the DVE min chain; emitting it after the NEXT sub-batch's
                # attention matmuls keeps the PE queue from ever waiting.
                xt, F, RS = state.pop(s)

                # export fused for the host-side topk mask correction
                # (all tail DMAs ride the Sync queue so their dispatch +
                # completion waits never block Scalar's evac/exp FIFO)
                nc.sync.dma_start(
                    out=fus_d[s, 0],
                    in_=F[0:N].rearrange("p j n -> p (j n)"))
                nc.sync.dma_start(
                    out=fus_d[s, 1],
                    in_=F[64:64 + N].rearrange("p j n -> p (j n)"))
                rs_dram = dpool.tile([2, 8, N], f32, tag="rsd",
                                     name=f"rsd_{s}")
                nc.sync.dma_start(
                    out=rs_dram[0].transpose([1, 0]), in_=RS[0:N, :])
                nc.sync.dma_start(
                    out=rs_dram[1].transpose([1, 0]), in_=RS[64:64 + N, :])
                Rs = spool.tile([SB, N], f32, tag="Rs", name=f"Rs_{s}")
                nc.sync.dma_start(
                    out=Rs[:],
                    in_=rs_dram[:].rearrange("k j n -> (k j) n"))
                Cp = cpspool.tile([2, HF], f32, tag="C", name=f"C_{s}")
                nc.tensor.matmul(
                    Cp[:], lhsT=ones2[:],
                    rhs=F[:].rearrange("p j n -> p (j n)"),
                    start=True, stop=True)
                Csb = spool.tile([2, 8, N], f32, tag="Csb", name=f"Csb_{s}")
                nc.scalar.copy(
                    out=Csb[:], in_=Cp[:].rearrange("p (j n) -> p j n", n=N))

                cs_dram = dpool.tile([2, 8, N], f32, tag="csd",
                                     name=f"csd_{s}")
                nc.sync.dma_start(out=cs_dram[:], in_=Csb[:])
                Cs = spool.tile([SB, N], f32, tag="Cs", name=f"Cs_{s}")
                nc.sync.dma_start(
                    out=Cs[:],
                    in_=cs_dram[:].rearrange("k j n -> (k j) n"))

                # ---- att + 1 = (colsum+1)/(49*(rowsum+1)) + 1 ----
                D = spool.tile([SB, N], f32, tag="D", name=f"D_{s}")
                nc.vector.tensor_scalar(D[:], Rs[:], float(N), float(N),
                                        op0=ALU.mult, op1=ALU.add)
                nc.vector.reciprocal(out=D[:], in_=D[:])
                M1 = spool.tile([SB, N], f32, tag="M1", name=f"M1_{s}")
                nc.vector.tensor_scalar_add(M1[:], Cs[:], 1.0)
                nc.vector.tensor_tensor(
                    out=M1[:], in0=M1[:], in1=D[:], op=ALU.mult)
                M1h = spool.tile([SB, N], bf16, tag="M1h", name=f"M1h_{s}")
                nc.vector.tensor_scalar_add(M1h[:], M1[:], 1.0)

                # broadcast multiplier to all 128 partitions via DRAM
                m1_dram = dpool.tile([FDX], bf16, tag="m1d", name=f"m1d_{s}")
                nc.sync.dma_start(out=m1_dram[:], in_=M1h[:])
                M1b = spool.tile([128, SB, N], bf16, tag="M1b",
                                 name=f"M1b_{s}")
                nc.sync.dma_start(
                    out=M1b[:],
                    in_=m1_dram[:].rearrange("(b n) -> b n",
                                             n=N).partition_broadcast(128))

                # ---- rx = x * (1 + att), in place (bf16), then store via
                # a casting DMA (bf16 -> fp32) on the gpsimd queue ----
                for k in range(CT):
                    eng = nc.vector if k < 3 else nc.gpsimd
                    eng.tensor_tensor(
                        out=xt[:, k], in0=xt[:, k], in1=M1b[:], op=ALU.mult)
                dst = rx_d.rearrange("(ct p) b n -> p ct b n",
                                     p=128)[:, :, SB * s:SB * (s + 1), :]
                nc.gpsimd.dma_start(out=dst, in_=xt[:])

            for s in range(nsb):
                emit_front(s)
                if s > 1:
                    emit_tail(s - 2)
                emit_attn(s)
            if nsb > 1:
                emit_tail(nsb - 2)
            emit_tail(nsb - 1)

    nc.compile()
    return nc


def _get_program(nsb=NSB):
    if nsb not in _CACHE:
        _CACHE[nsb] = _build(nsb)
    return _CACHE[nsb]


def _host_finalize(rx, x5, fused_all):
    """Exact replication of the reference's flat-topk masking quirk.

    Only global sample 0 is affected: its fused matrix is masked by the
    union of all samples' bottom-90% index sets (computed from the
    device-exported fused matrices), then its att row is rebuilt exactly.
    """
    thr = np.partition(fused_all, NN - KEEP, axis=1)[:, NN - KEEP]
    in_top = fused_all >= thr[:, None]
    zero_mask = (~in_top).any(axis=0)
    zero_mask[0] = False
    f0 = fused_all[0].copy()
    f0[zero_mask] = 0.0
    fm = f0.reshape(N, N)
    rowsum = fm.sum(axis=1)
    colsum = fm.sum(axis=0)
    att0 = (colsum + 1.0) / (N * (rowsum + 1.0))
    rx[0] = x5[0] * (1.0 + att0[None, :].astype(np.float32))
    return rx


def _par(fn, n):
    from concurrent.futures import ThreadPoolExecutor
    with ThreadPoolExecutor(max_workers=n) as ex:
        list(ex.map(fn, range(n)))


def _prep_w(W_qkv):
    """[2C, C] -> fp8 [128, KT, MT*128]: transpose, scale, pad k-tiles."""
    wt = np.zeros((128 * KT, MT * 128), np.float32)
    wt[:C] = np.asarray(W_qkv, dtype=np.float32)[:MT * 128].T * WSC
    np.clip(wt, -240.0, 240.0, out=wt)
    return np.ascontiguousarray(
        wt.reshape(KT, 128, MT * 128).transpose(1, 0, 2)
    ).astype(ml_dtypes.float8_e4m3)


def kernel(x, W_qkv):
    from concourse.bass_utils import run_bass_kernel_spmd

    nc = _get_program()
    x5 = np.asarray(x, dtype=np.float32).reshape(B_FULL, C, N)
    wt = _prep_w(W_qkv)

    # per-core channel-major shards [C, B_CORE, N]
    shards = [np.empty((C, B_CORE, N), np.float32) for _ in range(NCORES)]
    _par(lambda c: np.copyto(
        shards[c], x5[B_CORE * c:B_CORE * (c + 1)].transpose(1, 0, 2)), NCORES)

    in_maps = [{"x": shards[c], "wt": wt} for c in range(NCORES)]
    res = run_bass_kernel_spmd(nc, in_maps, core_ids=list(range(NCORES)))
    global LAST_RESULTS
    LAST_RESULTS = res

    rx = np.empty((B_FULL, C, N), np.float32)
    fused_all = np.empty((B_FULL, NN), np.float32)

    def _gather(c):
        out = res.results[c]
        rx[B_CORE * c:B_CORE * (c + 1)] = out["rx"].transpose(1, 0, 2)
        f = out["fus"].astype(np.float32).reshape(
            NSB, 2, N, 8, N).transpose(0, 1, 3, 2, 4)
        fused_all[B_CORE * c:B_CORE * (c + 1)] = f.reshape(B_CORE, NN)

    _par(_gather, NCORES)

    rx = _host_finalize(rx, x5, fused_all)
    return rx.reshape(B_FULL, C, 7, 7)


# revision 23
# speedup vs baseline: 1.5087x; 1.0524x over previous
"""Trainium2 Bass kernel: fused ViT-style attention rollout gating.

Math (per sample b):
  qkT[d, n]   = W_qk[d, :] @ x[b][:, n]          (d = 2*896: q rows then k rows)
  logits[h]   = qT_h.T @ kT_h                    ([49, 49] per head, K = head_dim = 128)
  attn[h]     = softmax(scale * logits[h])       (row-wise; no max-subtraction: |logits*scale| < 2)
  fused       = min_h attn[h]                    ([49, 49])
  rowsum[n]   = sum_m fused[n, m];  colsum[m] = sum_n fused[n, m]
  att[m]      = (colsum[m] + 1) / (49 * (rowsum[m] + 1))
  rx[b]       = x[b] * (1 + att)                 (broadcast over channels)

The reference's flat-topk masking quirk only touches global sample 0; it is
corrected exactly on the host from the device-exported `fused` matrices.

Sharding: pure data-parallel, 128 samples per core across 8 cores.
Layout per core/sub-batch (SB=16 samples): x is loaded as 7 c-tiles
[128c, 16b, 49n]; attention tiles pack 16 samples as 2 vertical partition
blocks (base 0 / 64, via PE column tiling) x 8 horizontal 49-col slots.

Projection runs in fp8 (e4m3) with MatmulPerfMode.DoubleRow: W is
pre-scaled by 32 on the host, cast to fp8, and contraction is padded
896 -> 1024 so each matmul contracts 256 rows (4 k-pair steps/output).
The 1/(32*32) de-scale folds exactly into the softmax exp scale.
"""

import numpy as np
import ml_dtypes

# ---- problem constants (hardcoded per contest rules) ----
B_FULL = 1024
C = 896
N = 49                   # tokens (7x7)
NH = 7                   # heads
HD = 128                 # head dim
NCORES = 8
B_CORE = B_FULL // NCORES   # 128
SB = 16                     # samples per sub-batch
NSB = B_CORE // SB          # 8 sub-batches
CT = C // 128               # 7 contraction tiles
KT = 8                      # padded contraction tiles (fp8 DoubleRow pairs)
MT = 2 * C // 128           # 14 output d-tiles (q then k)
HF = 8 * N                  # 392 = half free width (8 horizontal samples)
FDX = SB * N                # 784
WSC = 32.0                  # host-side W pre-scale (fp8 range/precision)
SCALE = float(HD) ** -0.5
NN = N * N                  # 2401
KEEP = NN - int(NN * 0.9)   # 241 largest kept out of topk(smallest 90%)

_CACHE = {}
LAST_RESULTS = None  # BassKernelResults of the most recent kernel() call


def _build(nsb=NSB):
    import concourse.tile as tile
    from concourse import bacc, mybir

    dt = mybir.dt
    f32 = dt.float32
    bf16 = dt.bfloat16
    f8 = dt.float8e4
    AF = mybir.ActivationFunctionType
    ALU = mybir.AluOpType
    AX = mybir.AxisListType
    DR = mybir.MatmulPerfMode.DoubleRow

    nc = bacc.Bacc("TRN2", target_bir_lowering=False, debug=False,
                   num_devices=NCORES)
    # x and rx travel in channel-major layout [C, B, N] (host transposes both
    # ways) so every DMA run is 16*49*4 = 3136 contiguous bytes
    x_d = nc.dram_tensor("x", [C, B_CORE, N], f32, kind="ExternalInput").ap()
    wt_d = nc.dram_tensor("wt", [128, KT, MT * 128], f8,
                          kind="ExternalInput").ap()
    rx_d = nc.dram_tensor("rx", [C, B_CORE, N], f32, kind="ExternalOutput").ap()
    fus_d = nc.dram_tensor("fus", [NSB, 2, N, HF], bf16,
                           kind="ExternalOutput").ap()

    with tile.TileContext(nc) as tc:
        with (
            tc.tile_pool(name="w", bufs=1) as wpool,
            tc.tile_pool(name="xt", bufs=6) as xtpool,
            tc.tile_pool(name="xb", bufs=2) as xbpool,
            tc.tile_pool(name="qk", bufs=2) as qkpool,
            tc.tile_pool(name="e", bufs=1) as epool,
            tc.tile_pool(name="sm", bufs=2) as spool,
            tc.tile_pool(name="qps", bufs=2, space="PSUM") as qpspool,
            tc.tile_pool(name="aps", bufs=3, space="PSUM") as apspool,
            tc.tile_pool(name="cps", bufs=1, space="PSUM") as cpspool,
            tc.tile_pool(name="dram", bufs=2, space="DRAM") as dpool,
        ):
            # ---- one-time: fp8 weights (pre-transposed/scaled/padded on
            # host, k-tile 7 all-zero) + colsum ones ----
            w8 = wpool.tile([128, KT, MT * 128], f8, tag="w8")
            nc.sync.dma_start(out=w8[:], in_=wt_d)
            ones2 = wpool.tile([128, 2], bf16, tag="ones2")
            nc.vector.memset(ones2[:], 0.0)
            nc.vector.memset(ones2[0:N, 0:1], 1.0)
            nc.vector.memset(ones2[64:64 + N, 1:2], 1.0)

            state = {}
            qkv_state = {}

            def emit_front(s):
                # ---- load x once (fp32->bf16 cast DMA), then bf16->fp8 ----
                x_src = x_d.rearrange("(ct p) b n -> p ct b n",
                                      p=128)[:, :, SB * s:SB * (s + 1), :]
                xt = xtpool.tile([128, CT, SB, N], bf16, tag="xt",
                                 name=f"xt_{s}")
                nc.gpsimd.dma_start(out=xt[:], in_=x_src)
                xb = xbpool.tile([128, KT, FDX], f8, tag="xb",
                                 name=f"xb_{s}")
                if s < 2:
                    # zero the contraction padding tile once per pool slot
                    # (bufs=2, tag-deterministic rotation)
                    nc.vector.memset(xb[:, CT, :], 0.0)
                # bf16 -> fp8 casts on Scalar: keeps them out of the DVE
                # FIFO, where they queued behind the softmax chain and
                # starved the projection matmuls
                for k in range(CT):
                    src = xt[:, k].rearrange("p b n -> p (b n)")
                    nc.scalar.copy(out=xb[:, k, :], in_=src)

                # ---- qkv projection (fp8 DoubleRow): qkT[m] = W[m] @ x ----
                # qk tiles carry 16 zero columns of tail padding so the
                # M=64-wide attention stationary reads below never run off
                # the end (jj=15 reads columns 735:799)
                qks = []
                for m in range(MT):
                    qk = qkpool.tile([128, FDX + 16], bf16, tag=f"qk{m}",
                                     name=f"qk{m}_{s}")
                    if s < 2:
                        # zero the stationary-overhang padding once per pool
                        # slot; nothing ever writes these 16 columns again
                        nc.vector.memset(qk[:, FDX:], 0.0)
                    q = qpspool.tile([128, 2, 512], f32, tag="qps",
                                     name=f"qps_{m}_{s}")
                    for half in range(2):
                        for kp in range(KT // 2):
                            nc.tensor.matmul(
                                q[:, half, 0:HF],
                                lhsT=w8[:, 2 * kp:2 * kp + 2,
                                        128 * m:128 * (m + 1)],
                                rhs=xb[:, 2 * kp:2 * kp + 2,
                                       HF * half:HF * (half + 1)],
                                start=(kp == 0), stop=(kp == KT // 2 - 1),
                                perf_mode=DR)
                    # single PSUM->SBUF evacuation per m on Scalar
                    # (GPSIMD cannot read PSUM)
                    dst = qk[:, 0:FDX].rearrange("p (h f) -> p h f", h=2)
                    nc.scalar.copy(out=dst, in_=q[:, :, 0:HF])
                    qks.append(qk)
                qkv_state[s] = (xt, qks)

            def emit_attn(s):
                xt, qks = qkv_state.pop(s)
                # ---- attention logits + exp + per-head normalize + min ----
                S = spool.tile([128, NH, 8], bf16, tag="S", name=f"S_{s}")
                R = spool.tile([128, NH, 8], bf16, tag="R", name=f"R_{s}")
                F = spool.tile([128, 8, N], bf16, tag="F", name=f"F_{s}",
                               bufs=3)
                Es = []
                for h in range(NH):
                    A = apspool.tile([128, HF], f32, tag="A", name=f"A{h}_{s}")
                    for j in range(8):
                        # vertical block 0: samples j     -> partitions 0:49
                        # vertical block 1: samples 8 + j -> partitions 64:113
                        # (M=64 stationary: rows 49:64 / 113:128 get junk dot
                        # products from neighboring columns — finite, masked
                        # out downstream — so every PSUM row is written)
                        nc.tensor.matmul(
                            A[0:64, N * j:N * (j + 1)],
                            lhsT=qks[h][:, N * j:N * j + 64],
                            rhs=qks[NH + h][:, N * j:N * (j + 1)],
                            start=True, stop=True)
                        nc.tensor.matmul(
                            A[64:128, N * j:N * (j + 1)],
                            lhsT=qks[h][:, N * (8 + j):N * (8 + j) + 64],
                            rhs=qks[NH + h][:, N * (8 + j):N * (9 + j)],
                            start=True, stop=True)
                    E = epool.tile([128, 8, N], bf16, tag=f"E{h}",
                                   name=f"E{h}_{s}")
                    nc.scalar.activation(
                        out=E[:], in_=A[:].rearrange("p (j n) -> p j n", n=N),
                        func=AF.Exp, scale=SCALE / (WSC * WSC))
                    with nc.allow_low_precision(
                            reason="49-term softmax sums; 2e-2 gate"):
                        nc.vector.reduce_sum(out=S[:, h, :], in_=E[:],
                                             axis=AX.X)
                        nc.vector.reciprocal(out=R[:, h, :], in_=S[:, h, :])
                    rb = R[:, h, :].unsqueeze(2).broadcast_to([128, 8, N])
                    nc.vector.tensor_tensor(
                        out=E[:], in0=E[:], in1=rb, op=ALU.mult)
                    Es.append(E)
                # min over heads as a depth-3 tree (shorter serial chain
                # after the last head's softmax than a running min)
                for a, b in ((0, 1), (2, 3), (4, 5), (0, 2), (4, 6)):
                    nc.vector.tensor_tensor(
                        out=Es[a][:], in0=Es[a][:], in1=Es[b][:], op=ALU.min)
                nc.vector.tensor_tensor(
                    out=F[:], in0=Es[0][:], in1=Es[4][:], op=ALU.min)

                # ---- rowsum (free-dim reduce); all dependent DMAs are
                # deferred to the tail so they never block a queue head ----
                RS = spool.tile([128, 8], f32, tag="RS", name=f"RS_{s}",
                                bufs=3)
                nc.vector.reduce_sum(out=RS[:], in_=F[:], axis=AX.X)
                state[s] = (xt, F, RS)

            def emit_tail(s):
                # Deferred by one sub-batch: the colsum matmul below depends
                # on the DVE min chain; emitting it after the NEXT sub-batch's
                # attention matmuls keeps the PE queue from ever waiting.
                xt, F, RS = state.pop(s)

                # export fused for the host-side topk mask correction
                # (all tail DMAs ride the Sync queue so their dispatch +
                # completion waits never block Scalar's evac/exp FIFO)
                nc.sync.dma_start(
                    out=fus_d[s, 0],
                    in_=F[0:N].rearrange("p j n -> p (j n)"))
                nc.sync.dma_start(
                    out=fus_d[s, 1],
                    in_=F[64:64 + N].rearrange("p j n -> p (j n)"))
                rs_dram = dpool.tile([2, 8, N], f32, tag="rsd",
                                     name=f"rsd_{s}")
                nc.sync.dma_start(
                    out=rs_dram[0].transpose([1, 0]), in_=RS[0:N, :])
                nc.sync.dma_start(
                    out=rs_dram[1].transpose([1, 0]), in_=RS[64:64 + N, :])
                Rs = spool.tile([SB, N], f32, tag="Rs", name=f"Rs_{s}")
                nc.sync.dma_start(
                    out=Rs[:],
                    in_=rs_dram[:].rearrange("k j n -> (k j) n"))
                Cp = cpspool.tile([2, HF], f32, tag="C", name=f"C_{s}")
                nc.tensor.matmul(
                    Cp[:], lhsT=ones2[:],
                    rhs=F[:].rearrange("p j n -> p (j n)"),
                    start=True, stop=True)
                Csb = spool.tile([2, 8, N], f32, tag="Csb", name=f"Csb_{s}")
                nc.scalar.copy(
                    out=Csb[:], in_=Cp[:].rearrange("p (j n) -> p j n", n=N))

                cs_dram = dpool.tile([2, 8, N], f32, tag="csd",
                                     name=f"csd_{s}")
                nc.sync.dma_start(out=cs_dram[:], in_=Csb[:])
                Cs = spool.tile([SB, N], f32, tag="Cs", name=f"Cs_{s}")
                nc.sync.dma_start(
                    out=Cs[:],
                    in_=cs_dram[:].rearrange("k j n -> (k j) n"))

                # ---- att + 1 = (colsum+1)/(49*(rowsum+1)) + 1 ----
                D = spool.tile([SB, N], f32, tag="D", name=f"D_{s}")
                nc.vector.tensor_scalar(D[:], Rs[:], float(N), float(N),
                                        op0=ALU.mult, op1=ALU.add)
                nc.vector.reciprocal(out=D[:], in_=D[:])
                M1 = spool.tile([SB, N], f32, tag="M1", name=f"M1_{s}")
                nc.vector.tensor_scalar_add(M1[:], Cs[:], 1.0)
                nc.vector.tensor_tensor(
                    out=M1[:], in0=M1[:], in1=D[:], op=ALU.mult)
                M1h = spool.tile([SB, N], bf16, tag="M1h", name=f"M1h_{s}")
                nc.vector.tensor_scalar_add(M1h[:], M1[:], 1.0)

                # broadcast multiplier to all 128 partitions via DRAM
                m1_dram = dpool.tile([FDX], bf16, tag="m1d", name=f"m1d_{s}")
                nc.sync.dma_start(out=m1_dram[:], in_=M1h[:])
                M1b = spool.tile([128, SB, N], bf16, tag="M1b",
                                 name=f"M1b_{s}")
                nc.sync.dma_start(
                    out=M1b[:],
                    in_=m1_dram[:].rearrange("(b n) -> b n",
                                             n=N).partition_broadcast(128))

                # ---- rx = x * (1 + att), in place (bf16), then store via
                # a casting DMA (bf16 -> fp32) on the gpsimd queue ----
                m1b = M1b[:].unsqueeze(1).broadcast_to([128, CT, SB, N])
                nc.vector.tensor_tensor(
                    out=xt[:], in0=xt[:], in1=m1b, op=ALU.mult)
                dst = rx_d.rearrange("(ct p) b n -> p ct b n",
                                     p=128)[:, :, SB * s:SB * (s + 1), :]
                nc.gpsimd.dma_start(out=dst, in_=xt[:])

            for s in range(nsb):
                emit_front(s)
                if s > 0:
                    emit_tail(s - 1)
                emit_attn(s)
            emit_tail(nsb - 1)

    nc.compile()
    return nc


def _get_program(nsb=NSB):
    if nsb not in _CACHE:
        _CACHE[nsb] = _build(nsb)
    return _CACHE[nsb]


def _host_finalize(rx, x5, fused_all):
    """Exact replication of the reference's flat-topk masking quirk.

    Only global sample 0 is affected: its fused matrix is masked by the
    union of all samples' bottom-90% index sets (computed from the
    device-exported fused matrices), then its att row is rebuilt exactly.
    """
    thr = np.partition(fused_all, NN - KEEP, axis=1)[:, NN - KEEP]
    in_top = fused_all >= thr[:, None]
    zero_mask = (~in_top).any(axis=0)
    zero_mask[0] = False
    f0 = fused_all[0].copy()
    f0[zero_mask] = 0.0
    fm = f0.reshape(N, N)
    rowsum = fm.sum(axis=1)
    colsum = fm.sum(axis=0)
    att0 = (colsum + 1.0) / (N * (rowsum + 1.0))
    rx[0] = x5[0] * (1.0 + att0[None, :].astype(np.float32))
    return rx


def _par(fn, n):
    from concurrent.futures import ThreadPoolExecutor
    with ThreadPoolExecutor(max_workers=n) as ex:
        list(ex.map(fn, range(n)))


def _prep_w(W_qkv):
    """[2C, C] -> fp8 [128, KT, MT*128]: transpose, scale, pad k-tiles."""
    wt = np.zeros((128 * KT, MT * 128), np.float32)
    wt[:C] = np.asarray(W_qkv, dtype=np.float32)[:MT * 128].T * WSC
    np.clip(wt, -240.0, 240.0, out=wt)
    return np.ascontiguousarray(
        wt.reshape(KT, 128, MT * 128).transpose(1, 0, 2)
    ).astype(ml_dtypes.float8_e4m3)


def kernel(x, W_qkv):
    from concourse.bass_utils import run_bass_kernel_spmd

    nc = _get_program()
    x5 = np.asarray(x, dtype=np.float32).reshape(B_FULL, C, N)
    wt = _prep_w(W_qkv)

    # per-core channel-major shards [C, B_CORE, N]
    shards = [np.empty((C, B_CORE, N), np.float32) for _ in range(NCORES)]
    _par(lambda c: np.copyto(
        shards[c], x5[B_CORE * c:B_CORE * (c + 1)].transpose(1, 0, 2)), NCORES)

    in_maps = [{"x": shards[c], "wt": wt} for c in range(NCORES)]
    res = run_bass_kernel_spmd(nc, in_maps, core_ids=list(range(NCORES)))
    global LAST_RESULTS
    LAST_RESULTS = res

    rx = np.empty((B_FULL, C, N), np.float32)
    fused_all = np.empty((B_FULL, NN), np.float32)

    def _gather(c):
        out = res.results[c]
        rx[B_CORE * c:B_CORE * (c + 1)] = out["rx"].transpose(1, 0, 2)
        f = out["fus"].astype(np.float32).reshape(
            NSB, 2, N, 8, N).transpose(0, 1, 3, 2, 4)
        fused_all[B_CORE * c:B_CORE * (c + 1)] = f.reshape(B_CORE, NN)

    _par(_gather, NCORES)

    rx = _host_finalize(rx, x5, fused_all)
    return rx.reshape(B_FULL, C, 7, 7)


# revision 27
# speedup vs baseline: 1.6593x; 1.0998x over previous
"""Trainium2 Bass kernel: fused ViT-style attention rollout gating.

Math (per sample b):
  qkT[d, n]   = W_qk[d, :] @ x[b][:, n]          (d = 2*896: q rows then k rows)
  logits[h]   = qT_h.T @ kT_h                    ([49, 49] per head, K = head_dim = 128)
  attn[h]     = softmax(scale * logits[h])       (row-wise; no max-subtraction: |logits*scale| < 2)
  fused       = min_h attn[h]                    ([49, 49])
  rowsum[n]   = sum_m fused[n, m];  colsum[m] = sum_n fused[n, m]
  att[m]      = (colsum[m] + 1) / (49 * (rowsum[m] + 1))
  rx[b]       = x[b] * (1 + att)                 (broadcast over channels)

The reference's flat-topk masking quirk only touches global sample 0; it is
corrected exactly on the host from the device-exported `fused` matrices.

Sharding: pure data-parallel, 128 samples per core across 8 cores.
Layout per core/sub-batch (SB=16 samples): x is loaded as 7 c-tiles
[128c, 16b, 49n]; attention tiles pack 16 samples as 2 vertical partition
blocks (base 0 / 64, via PE column tiling) x 8 horizontal 49-col slots.

Projection runs in fp8 (e4m3) with MatmulPerfMode.DoubleRow: W is
pre-scaled by 32 on the host, cast to fp8, and contraction is padded
896 -> 1024 so each matmul contracts 256 rows (4 k-pair steps/output).
The 1/(32*32) de-scale folds exactly into the softmax exp scale.
"""

import numpy as np
import ml_dtypes

# ---- problem constants (hardcoded per contest rules) ----
B_FULL = 1024
C = 896
N = 49                   # tokens (7x7)
NH = 7                   # heads
HD = 128                 # head dim
NCORES = 8
B_CORE = B_FULL // NCORES   # 128
SB = 16                     # samples per sub-batch
NSB = B_CORE // SB          # 8 sub-batches
CT = C // 128               # 7 contraction tiles
KT = 8                      # padded contraction tiles (fp8 DoubleRow pairs)
MT = 2 * C // 128           # 14 output d-tiles (q then k)
HF = 8 * N                  # 392 = half free width (8 horizontal samples)
FDX = SB * N                # 784
WSC = 32.0                  # host-side W pre-scale (fp8 range/precision)
SCALE = float(HD) ** -0.5
NN = N * N                  # 2401
KEEP = NN - int(NN * 0.9)   # 241 largest kept out of topk(smallest 90%)

_CACHE = {}
LAST_RESULTS = None  # BassKernelResults of the most recent kernel() call


def _build(nsb=NSB):
    import concourse.tile as tile
    from concourse import bacc, mybir

    dt = mybir.dt
    f32 = dt.float32
    bf16 = dt.bfloat16
    f8 = dt.float8e4
    AF = mybir.ActivationFunctionType
    ALU = mybir.AluOpType
    AX = mybir.AxisListType
    DR = mybir.MatmulPerfMode.DoubleRow

    nc = bacc.Bacc("TRN2", target_bir_lowering=False, debug=False,
                   num_devices=NCORES)
    # x and rx travel in channel-major layout [C, B, N] (host transposes both
    # ways) so every DMA run is 16*49*4 = 3136 contiguous bytes
    x_d = nc.dram_tensor("x", [C, B_CORE, N], f32, kind="ExternalInput").ap()
    wt_d = nc.dram_tensor("wt", [128, KT, MT * 128], f8,
                          kind="ExternalInput").ap()
    rx_d = nc.dram_tensor("rx", [C, B_CORE, N], f32, kind="ExternalOutput").ap()
    fus_d = nc.dram_tensor("fus", [NSB, 2, N, HF], bf16,
                           kind="ExternalOutput").ap()

    with tile.TileContext(nc) as tc:
        with (
            tc.tile_pool(name="w", bufs=1) as wpool,
            tc.tile_pool(name="xt", bufs=6) as xtpool,
            tc.tile_pool(name="xb", bufs=2) as xbpool,
            tc.tile_pool(name="qk", bufs=2) as qkpool,
            tc.tile_pool(name="e", bufs=1) as epool,
            tc.tile_pool(name="sm", bufs=2) as spool,
            tc.tile_pool(name="qps", bufs=4, space="PSUM") as qpspool,
            tc.tile_pool(name="aps", bufs=3, space="PSUM") as apspool,
            tc.tile_pool(name="cps", bufs=1, space="PSUM") as cpspool,
            tc.tile_pool(name="dram", bufs=2, space="DRAM") as dpool,
        ):
            # ---- one-time: fp8 weights (pre-transposed/scaled/padded on
            # host, k-tile 7 all-zero) + colsum ones ----
            w8 = wpool.tile([128, KT, MT * 128], f8, tag="w8")
            nc.sync.dma_start(out=w8[:], in_=wt_d)
            ones2 = wpool.tile([128, 2], bf16, tag="ones2")
            nc.vector.memset(ones2[:], 0.0)
            nc.vector.memset(ones2[0:N, 0:1], 1.0)
            nc.vector.memset(ones2[64:64 + N, 1:2], 1.0)

            state = {}
            qkv_state = {}
            load_state = {}

            def emit_load(s):
                # ---- load x once (fp32->bf16 cast DMA) ----
                x_src = x_d.rearrange("(ct p) b n -> p ct b n",
                                      p=128)[:, :, SB * s:SB * (s + 1), :]
                xt = xtpool.tile([128, CT, SB, N], bf16, tag="xt",
                                 name=f"xt_{s}")
                nc.gpsimd.dma_start(out=xt[:], in_=x_src)
                load_state[s] = xt

            def emit_cast(s):
                # bf16 -> fp8 casts on DVE, emitted one sub-batch ahead of
                # the softmax chain so the projection never waits on them
                xt = load_state[s]
                xb = xbpool.tile([128, KT, FDX], f8, tag="xb",
                                 name=f"xb_{s}")
                if s < 2:
                    # zero the contraction padding tile once per pool slot
                    # (bufs=2, tag-deterministic rotation)
                    nc.vector.memset(xb[:, CT, :], 0.0)
                for k in range(CT):
                    src = xt[:, k].rearrange("p b n -> p (b n)")
                    nc.vector.tensor_copy(out=xb[:, k, :], in_=src)
                load_state[s] = (xt, xb)

            def emit_proj(s):
                xt, xb = load_state.pop(s)
                # ---- qkv projection (fp8 DoubleRow): qkT[m] = W[m] @ x ----
                # qk tiles carry 16 zero columns of tail padding so the
                # M=64-wide attention stationary reads below never run off
                # the end (jj=15 reads columns 735:799)
                # PSUM: one bank per (m, half) group, 4-deep rotation, so
                # the group-start WAR wait (moved onto LDWEIGHTS) lands 3
                # groups back and never fences the weight-load pull-ahead
                qks = []
                for m in range(MT):
                    qk = qkpool.tile([128, FDX + 16], bf16, tag=f"qk{m}",
                                     name=f"qk{m}_{s}")
                    if s < 2:
                        # zero the stationary-overhang padding once per pool
                        # slot; nothing ever writes these 16 columns again
                        nc.vector.memset(qk[:, FDX:], 0.0)
                    for half in range(2):
                        q = qpspool.tile([128, 512], f32, tag="qps",
                                         name=f"qps_{m}_{half}_{s}")
                        for kp in range(KT // 2):
                            nc.tensor.matmul(
                                q[:, 0:HF],
                                lhsT=w8[:, 2 * kp:2 * kp + 2,
                                        128 * m:128 * (m + 1)],
                                rhs=xb[:, 2 * kp:2 * kp + 2,
                                       HF * half:HF * (half + 1)],
                                start=(kp == 0), stop=(kp == KT // 2 - 1),
                                perf_mode=DR)
                        # per-half PSUM->SBUF evacuation on Scalar
                        # (GPSIMD cannot read PSUM)
                        nc.scalar.copy(
                            out=qk[:, HF * half:HF * (half + 1)],
                            in_=q[:, 0:HF])
                    qks.append(qk)
                qkv_state[s] = (xt, qks)

            def emit_attn(s):
                xt, qks = qkv_state.pop(s)
                # ---- attention logits + exp + per-head normalize + min ----
                S = spool.tile([128, NH, 8], bf16, tag="S", name=f"S_{s}")
                R = spool.tile([128, NH, 8], bf16, tag="R", name=f"R_{s}")
                F = spool.tile([128, 8, N], bf16, tag="F", name=f"F_{s}",
                               bufs=3)
                Es = []
                for h in range(NH):
                    A = apspool.tile([128, HF], f32, tag="A", name=f"A{h}_{s}")
                    for j in range(8):
                        # vertical block 0: samples j     -> partitions 0:49
                        # vertical block 1: samples 8 + j -> partitions 64:113
                        # (M=64 stationary: rows 49:64 / 113:128 get junk dot
                        # products from neighboring columns — finite, masked
                        # out downstream — so every PSUM row is written)
                        nc.tensor.matmul(
                            A[0:64, N * j:N * (j + 1)],
                            lhsT=qks[h][:, N * j:N * j + 64],
                            rhs=qks[NH + h][:, N * j:N * (j + 1)],
                            start=True, stop=True)
                        nc.tensor.matmul(
                            A[64:128, N * j:N * (j + 1)],
                            lhsT=qks[h][:, N * (8 + j):N * (8 + j) + 64],
                            rhs=qks[NH + h][:, N * (8 + j):N * (9 + j)],
                            start=True, stop=True)
                    E = epool.tile([128, 8, N], bf16, tag=f"E{h}",
                                   name=f"E{h}_{s}")
                    nc.scalar.activation(
                        out=E[:], in_=A[:].rearrange("p (j n) -> p j n", n=N),
                        func=AF.Exp, scale=SCALE / (WSC * WSC))
                    with nc.allow_low_precision(
                            reason="49-term softmax sums; 2e-2 gate"):
                        nc.vector.reduce_sum(out=S[:, h, :], in_=E[:],
                                             axis=AX.X)
                        nc.vector.reciprocal(out=R[:, h, :], in_=S[:, h, :])
                    rb = R[:, h, :].unsqueeze(2).broadcast_to([128, 8, N])
                    nc.vector.tensor_tensor(
                        out=E[:], in0=E[:], in1=rb, op=ALU.mult)
                    Es.append(E)
                # min over heads as a depth-3 tree (shorter serial chain
                # after the last head's softmax than a running min)
                for a, b in ((0, 1), (2, 3), (4, 5), (0, 2), (4, 6)):
                    nc.vector.tensor_tensor(
                        out=Es[a][:], in0=Es[a][:], in1=Es[b][:], op=ALU.min)
                nc.vector.tensor_tensor(
                    out=F[:], in0=Es[0][:], in1=Es[4][:], op=ALU.min)

                # ---- rowsum (free-dim reduce); all dependent DMAs are
                # deferred to the tail so they never block a queue head ----
                RS = spool.tile([128, 8], f32, tag="RS", name=f"RS_{s}",
                                bufs=3)
                nc.vector.reduce_sum(out=RS[:], in_=F[:], axis=AX.X)
                state[s] = (xt, F, RS)

            def emit_tail(s):
                # Deferred by one sub-batch: the colsum matmul below depends
                # on the DVE min chain; emitting it after the NEXT sub-batch's
                # attention matmuls keeps the PE queue from ever waiting.
                xt, F, RS = state.pop(s)

                # export fused for the host-side topk mask correction
                # (all tail DMAs ride the Sync queue so their dispatch +
                # completion waits never block Scalar's evac/exp FIFO)
                nc.sync.dma_start(
                    out=fus_d[s, 0],
                    in_=F[0:N].rearrange("p j n -> p (j n)"))
                nc.sync.dma_start(
                    out=fus_d[s, 1],
                    in_=F[64:64 + N].rearrange("p j n -> p (j n)"))
                rs_dram = dpool.tile([2, 8, N], f32, tag="rsd",
                                     name=f"rsd_{s}")
                nc.sync.dma_start(
                    out=rs_dram[0].transpose([1, 0]), in_=RS[0:N, :])
                nc.sync.dma_start(
                    out=rs_dram[1].transpose([1, 0]), in_=RS[64:64 + N, :])
                Rs = spool.tile([SB, N], f32, tag="Rs", name=f"Rs_{s}")
                nc.sync.dma_start(
                    out=Rs[:],
                    in_=rs_dram[:].rearrange("k j n -> (k j) n"))
                Cp = cpspool.tile([2, HF], f32, tag="C", name=f"C_{s}")
                nc.tensor.matmul(
                    Cp[:], lhsT=ones2[:],
                    rhs=F[:].rearrange("p j n -> p (j n)"),
                    start=True, stop=True)
                Csb = spool.tile([2, 8, N], f32, tag="Csb", name=f"Csb_{s}")
                nc.scalar.copy(
                    out=Csb[:], in_=Cp[:].rearrange("p (j n) -> p j n", n=N))

                cs_dram = dpool.tile([2, 8, N], f32, tag="csd",
                                     name=f"csd_{s}")
                nc.sync.dma_start(out=cs_dram[:], in_=Csb[:])
                Cs = spool.tile([SB, N], f32, tag="Cs", name=f"Cs_{s}")
                nc.sync.dma_start(
                    out=Cs[:],
                    in_=cs_dram[:].rearrange("k j n -> (k j) n"))

                # ---- att + 1 = (colsum+1)/(49*(rowsum+1)) + 1 ----
                D = spool.tile([SB, N], f32, tag="D", name=f"D_{s}")
                nc.vector.tensor_scalar(D[:], Rs[:], float(N), float(N),
                                        op0=ALU.mult, op1=ALU.add)
                nc.vector.reciprocal(out=D[:], in_=D[:])
                M1 = spool.tile([SB, N], f32, tag="M1", name=f"M1_{s}")
                nc.vector.tensor_scalar_add(M1[:], Cs[:], 1.0)
                nc.vector.tensor_tensor(
                    out=M1[:], in0=M1[:], in1=D[:], op=ALU.mult)
                M1h = spool.tile([SB, N], bf16, tag="M1h", name=f"M1h_{s}")
                nc.vector.tensor_scalar_add(M1h[:], M1[:], 1.0)

                # broadcast multiplier to all 128 partitions via DRAM
                m1_dram = dpool.tile([FDX], bf16, tag="m1d", name=f"m1d_{s}")
                nc.sync.dma_start(out=m1_dram[:], in_=M1h[:])
                M1b = spool.tile([128, SB, N], bf16, tag="M1b",
                                 name=f"M1b_{s}")
                nc.sync.dma_start(
                    out=M1b[:],
                    in_=m1_dram[:].rearrange("(b n) -> b n",
                                             n=N).partition_broadcast(128))

                # ---- rx = x * (1 + att), in place (bf16), then store via
                # a casting DMA (bf16 -> fp32) on the gpsimd queue ----
                m1b = M1b[:].unsqueeze(1).broadcast_to([128, CT, SB, N])
                nc.gpsimd.tensor_tensor(
                    out=xt[:], in0=xt[:], in1=m1b, op=ALU.mult)
                dst = rx_d.rearrange("(ct p) b n -> p ct b n",
                                     p=128)[:, :, SB * s:SB * (s + 1), :]
                nc.gpsimd.dma_start(out=dst, in_=xt[:])

            emit_load(0)
            emit_cast(0)
            for s in range(nsb):
                if s + 1 < nsb:
                    emit_load(s + 1)
                emit_proj(s)
                if s + 1 < nsb:
                    emit_cast(s + 1)
                if s > 0:
                    emit_tail(s - 1)
                emit_attn(s)
            emit_tail(nsb - 1)

    nc.compile()
    return nc


def _get_program(nsb=NSB):
    if nsb not in _CACHE:
        _CACHE[nsb] = _build(nsb)
    return _CACHE[nsb]


def _host_finalize(rx, x5, fused_all):
    """Exact replication of the reference's flat-topk masking quirk.

    Only global sample 0 is affected: its fused matrix is masked by the
    union of all samples' bottom-90% index sets (computed from the
    device-exported fused matrices), then its att row is rebuilt exactly.
    """
    thr = np.partition(fused_all, NN - KEEP, axis=1)[:, NN - KEEP]
    in_top = fused_all >= thr[:, None]
    zero_mask = (~in_top).any(axis=0)
    zero_mask[0] = False
    f0 = fused_all[0].copy()
    f0[zero_mask] = 0.0
    fm = f0.reshape(N, N)
    rowsum = fm.sum(axis=1)
    colsum = fm.sum(axis=0)
    att0 = (colsum + 1.0) / (N * (rowsum + 1.0))
    rx[0] = x5[0] * (1.0 + att0[None, :].astype(np.float32))
    return rx


def _par(fn, n):
    from concurrent.futures import ThreadPoolExecutor
    with ThreadPoolExecutor(max_workers=n) as ex:
        list(ex.map(fn, range(n)))


def _prep_w(W_qkv):
    """[2C, C] -> fp8 [128, KT, MT*128]: transpose, scale, pad k-tiles."""
    wt = np.zeros((128 * KT, MT * 128), np.float32)
    wt[:C] = np.asarray(W_qkv, dtype=np.float32)[:MT * 128].T * WSC
    np.clip(wt, -240.0, 240.0, out=wt)
    return np.ascontiguousarray(
        wt.reshape(KT, 128, MT * 128).transpose(1, 0, 2)
    ).astype(ml_dtypes.float8_e4m3)


def kernel(x, W_qkv):
    from concourse.bass_utils import run_bass_kernel_spmd

    nc = _get_program()
    x5 = np.asarray(x, dtype=np.float32).reshape(B_FULL, C, N)
    wt = _prep_w(W_qkv)

    # per-core channel-major shards [C, B_CORE, N]
    shards = [np.empty((C, B_CORE, N), np.float32) for _ in range(NCORES)]
    _par(lambda c: np.copyto(
        shards[c], x5[B_CORE * c:B_CORE * (c + 1)].transpose(1, 0, 2)), NCORES)

    in_maps = [{"x": shards[c], "wt": wt} for c in range(NCORES)]
    res = run_bass_kernel_spmd(nc, in_maps, core_ids=list(range(NCORES)))
    global LAST_RESULTS
    LAST_RESULTS = res

    rx = np.empty((B_FULL, C, N), np.float32)
    fused_all = np.empty((B_FULL, NN), np.float32)

    def _gather(c):
        out = res.results[c]
        rx[B_CORE * c:B_CORE * (c + 1)] = out["rx"].transpose(1, 0, 2)
        f = out["fus"].astype(np.float32).reshape(
            NSB, 2, N, 8, N).transpose(0, 1, 3, 2, 4)
        fused_all[B_CORE * c:B_CORE * (c + 1)] = f.reshape(B_CORE, NN)

    _par(_gather, NCORES)

    rx = _host_finalize(rx, x5, fused_all)
    return rx.reshape(B_FULL, C, 7, 7)
